# revision 1
# baseline (speedup 1.0000x reference)
"""Trainium2 Bass kernel for a 2-layer GAT + global mean pool + linear head.

Math (matches PyG GATConv, eval mode, single head, add_self_loops=True):
  h   = x @ W
  e_k = lrelu(ss[src_k] + sd[dst_k]),  ss = h@a_src, sd = h@a_dst
  alpha = softmax over incoming edges of each dst (self-loop included)
  out[d] = sum_k alpha_k h[src_k] + b
Two GAT layers (512->128, 128->64) with ReLU, then per-graph mean pool
over `batch` and a final [64,2] linear.

Strategy (8 NeuronCores, full inputs in / full output out):
  * Destination nodes sharded across cores (2500/core), sources arbitrary.
  * Each core computes h for its shard, assembles a gather table row
    [h | ss | pad] per node, AllGathers the table into every core's HBM.
  * Edges grouped per destination into fixed "slots" (padded with a
    sentinel table row that contributes ~0 to the softmax) and laid out
    destination-per-partition. Slot source rows are fetched with one
    indirect DMA per (block, sub-block).
  * Attention: ACT Lrelu (bias = per-partition sd) + ACT Exp with
    accum_out = softmax denominator. Aggregation: DVE broadcast-multiply
    + strided tensor_reduce. Dense matmuls/transposes/pooling on PE.
  * Per-graph pooling one-hots (with 1/count folded in) are host-built;
    partial pooled features are AllReduced, final linear on every core.

All graph-structure preprocessing (degree sort, slot layout, index
remapping) is host-side numpy on the kernel inputs; the device only sees
dense arrays.
"""

import math
import numpy as np

import concourse.bass as bass
import concourse.bacc as bacc
import concourse.mybir as mybir
from concourse.tile import TileContext
from concourse.masks import make_identity
from concourse.bass_utils import run_bass_kernel_spmd

F32 = mybir.dt.float32
BF16 = mybir.dt.bfloat16
I32 = mybir.dt.int32
AF = mybir.ActivationFunctionType
ALU = mybir.AluOpType

NEG_SLOPE = 0.2
SENT_SS = -60.0  # sentinel row score: exp(lrelu(-60+sd)) ~ e^-10 -> harmless


def full_cfg():
    return dict(N=20000, IND=512, HID=128, HID2=64, OUT=2, G=16, NCORES=8,
                LCAP=48)


# ----------------------------------------------------------------------------
# Host-side preprocessing
# ----------------------------------------------------------------------------

def preprocess(x, edge_index, batch, W1, a1_src, a1_dst, b1,
               W2, a2_src, a2_dst, b2, Wl, bl, cfg):
    N, IND, HID, HID2, OUT, G, NC = (cfg[k] for k in
                                     ("N", "IND", "HID", "HID2", "OUT", "G",
                                      "NCORES"))
    PC = math.ceil(N / NC)            # real dests per core
    PB = math.ceil(PC / 128)          # dest blocks per core
    PCP = PB * 128                    # padded dests per core
    TR = NC * PCP + 1                 # table rows (+1 sentinel)
    SENT = TR - 1
    R1 = HID + 1       # bf16 table row: [h | ss]
    R2 = HID2 + 1

    x = np.asarray(x, np.float32)
    batch = np.asarray(batch, np.int64)
    src = np.asarray(edge_index[0], np.int64)
    dst = np.asarray(edge_index[1], np.int64)
    # self loops
    loop = np.arange(N, dtype=np.int64)
    src = np.concatenate([src, loop])
    dst = np.concatenate([dst, loop])

    counts = np.bincount(batch, minlength=G).astype(np.float64)

    # per-core degree-sorted permutations and global row ids
    row_of = np.empty(N, np.int64)       # global node -> table row
    orders = []
    degs_sorted = np.zeros((NC, PCP), np.int64)
    core_of_dst = np.minimum(dst // PC, NC - 1)
    for k in range(NC):
        lo, hi = k * PC, min((k + 1) * PC, N)
        nk = hi - lo
        mask = (dst >= lo) & (dst < hi)
        deg = np.bincount(dst[mask] - lo, minlength=nk)
        order = np.argsort(-deg, kind="stable")        # local rank -> local id
        inv = np.empty(nk, np.int64)
        inv[order] = np.arange(nk)
        row_of[lo:hi] = k * PCP + inv
        orders.append(order)
        degs_sorted[k, :nk] = deg[order]

    # global per-block slot counts (identical program on every core)
    Ls = []
    for b in range(PB):
        Lb = int(degs_sorted[:, b * 128:(b + 1) * 128].max())
        Ls.append(max(Lb, 1))
    S = int(np.sum(Ls))
    offs = np.concatenate([[0], np.cumsum(Ls)]).astype(np.int64)

    # shared (replicated) weight uploads
    KB = IND // 128
    W1u = np.ascontiguousarray(W1.astype(np.float32).reshape(KB, 128, HID))
    W2u = np.ascontiguousarray(W2.astype(np.float32))
    a1s = np.tile(np.asarray(a1_src, np.float32)[None, :], (128, 1))
    a1d = np.tile(np.asarray(a1_dst, np.float32)[None, :], (128, 1))
    b1r = np.tile(np.asarray(b1, np.float32)[None, :], (128, 1))
    a2s = np.tile(np.asarray(a2_src, np.float32)[None, :], (128, 1))
    a2d = np.tile(np.asarray(a2_dst, np.float32)[None, :], (128, 1))
    b2r = np.tile(np.asarray(b2, np.float32)[None, :], (128, 1))
    WlBl = np.concatenate([np.asarray(Wl, np.float32),
                           np.asarray(bl, np.float32)[None, :]], axis=0)
    import ml_dtypes
    sent1 = np.zeros((1, R1), ml_dtypes.bfloat16)
    sent1[0, HID] = SENT_SS
    sent2 = np.zeros((1, R2), ml_dtypes.bfloat16)
    sent2[0, HID2] = SENT_SS

    in_maps = []
    for k in range(NC):
        lo, hi = k * PC, min((k + 1) * PC, N)
        nk = hi - lo
        order = orders[k]

        # xT: [KB, 128, PCP] (feature-major columns in local-rank order)
        xs = np.zeros((PCP, IND), np.float32)
        xs[:nk] = x[lo:hi][order]
        xT = np.ascontiguousarray(xs.T.reshape(KB, 128, PCP))

        # slot indices [128, S] -> table rows, sentinel padded
        sidx = np.full((128, S), SENT, np.int64)
        mask = (dst >= lo) & (dst < hi)
        es, ed = src[mask], dst[mask] - lo
        o = np.argsort(ed, kind="stable")
        es, ed = es[o], ed[o]
        deg = np.bincount(ed, minlength=nk)
        start = np.concatenate([[0], np.cumsum(deg)[:-1]])
        j = np.arange(len(ed)) - start[ed]            # slot within dest
        inv = np.empty(nk, np.int64)
        inv[order] = np.arange(nk)
        r = inv[ed]                                   # dest rank
        bb, pp = r // 128, r % 128
        col = offs[bb] + j
        sidx[pp, col] = row_of[es]
        idxw = sidx.astype(np.int32)

        # pooling one-hot with 1/count folded, zero rows for pad dests
        P = np.zeros((128, PB * G), np.float32)
        bg = batch[lo:hi][order]                      # graph id per rank
        rr = np.arange(nk)
        P[rr % 128, (rr // 128) * G + bg] = 1.0 / np.maximum(counts[bg], 1.0)

        in_maps.append(dict(
            xT=xT, W1u=W1u, W2u=W2u, a1s=a1s, a1d=a1d, b1r=b1r,
            a2s=a2s, a2d=a2d, b2r=b2r, WlBl=WlBl.astype(np.float32),
            Pp=P, sidx=idxw, sent1=sent1, sent2=sent2,
        ))

    meta = dict(PC=PC, PB=PB, PCP=PCP, TR=TR, R1=R1, R2=R2, KB=KB, S=S,
                Ls=Ls, offs=offs)
    return in_maps, meta


# ----------------------------------------------------------------------------
# Device program
# ----------------------------------------------------------------------------

def build_program(cfg, meta, debug_outs=False):
    N, IND, HID, HID2, OUT, G, NC, LCAP = (cfg[k] for k in
                                           ("N", "IND", "HID", "HID2", "OUT",
                                            "G", "NCORES", "LCAP"))
    PB, PCP, TR, R1, R2, KB, S = (meta[k] for k in
                                  ("PB", "PCP", "TR", "R1", "R2", "KB", "S"))

    Ls, offs = meta["Ls"], meta["offs"]

    nc = bacc.Bacc("TRN2", target_bir_lowering=False, debug=False,
                   num_devices=NC)

    xT_d = nc.declare_dram_parameter("xT", [KB, 128, PCP], F32, False)
    W1_d = nc.declare_dram_parameter("W1u", [KB, 128, HID], F32, False)
    W2_d = nc.declare_dram_parameter("W2u", [HID, HID2], F32, False)
    a1s_d = nc.declare_dram_parameter("a1s", [128, HID], F32, False)
    a1d_d = nc.declare_dram_parameter("a1d", [128, HID], F32, False)
    b1r_d = nc.declare_dram_parameter("b1r", [128, HID], F32, False)
    a2s_d = nc.declare_dram_parameter("a2s", [128, HID2], F32, False)
    a2d_d = nc.declare_dram_parameter("a2d", [128, HID2], F32, False)
    b2r_d = nc.declare_dram_parameter("b2r", [128, HID2], F32, False)
    Wl_d = nc.declare_dram_parameter("WlBl", [HID2 + 1, OUT], F32, False)
    Pp_d = nc.declare_dram_parameter("Pp", [128, PB * G], F32, False)
    sidx_d = nc.declare_dram_parameter("sidx", [128, S], I32, False)
    sent1_d = nc.declare_dram_parameter("sent1", [1, R1], BF16, False)
    sent2_d = nc.declare_dram_parameter("sent2", [1, R2], BF16, False)
    out_d = nc.declare_dram_parameter("out", [G, OUT], F32, True)
    if debug_outs:
        dbg_t1 = nc.declare_dram_parameter("dbg_t1", [TR, R1], BF16, True)
        dbg_sd1 = nc.declare_dram_parameter("dbg_sd1", [128, PB], F32, True)
        dbg_r2 = nc.declare_dram_parameter("dbg_r2", [PB * 128, HID2], F32,
                                           True)
        dbg_r1 = nc.declare_dram_parameter("dbg_r1", [PB * 128, HID], F32,
                                           True)
        dbg_g = nc.declare_dram_parameter("dbg_g", [128, LCAP * R1], BF16,
                                          True)
        dbg_den = nc.declare_dram_parameter("dbg_den", [128, PB], F32, True)

    shared = dict(addr_space="Shared") if NC > 4 else {}
    T1shard = nc.dram_tensor("T1shard", [PCP, R1], BF16)
    T1full = nc.dram_tensor("T1full", [TR, R1], BF16, **shared)
    T2shard = nc.dram_tensor("T2shard", [PCP, R2], BF16)
    T2full = nc.dram_tensor("T2full", [TR, R2], BF16, **shared)
    poolin = nc.dram_tensor("poolin", [G, HID2], F32)
    poolout = nc.dram_tensor("poolout", [G, HID2], F32, **shared)

    groups = [list(range(NC))]

    def subs_of(L):
        return [(s, min(LCAP, L - s)) for s in range(0, L, LCAP)]

    with TileContext(nc) as tc:
        with (
            tc.tile_pool(name="const", bufs=1) as cp,
            tc.tile_pool(name="work", bufs=3) as wp,
            tc.tile_pool(name="gath", bufs=2) as gp,
            tc.tile_pool(name="wtp", bufs=2) as wtp,
            tc.tile_pool(name="psA", bufs=2, space="PSUM") as psA,
            tc.tile_pool(name="psB", bufs=2, space="PSUM") as psB,
            tc.tile_pool(name="psP", bufs=1, space="PSUM") as psP,
        ):
            # ---------------- constants to SBUF ----------------
            W1_sb = cp.tile([128, KB * HID], F32, tag="w1")
            W1v = W1_sb[:].rearrange("p (k h) -> p k h", h=HID)
            nc.sync.dma_start(out=W1v, in_=W1_d[:].rearrange("k p h -> p k h"))
            W2_sb = cp.tile([HID, HID2], F32, tag="w2")
            nc.sync.dma_start(out=W2_sb[:], in_=W2_d[:])
            a1s_sb = cp.tile([128, HID], F32, tag="a1s")
            nc.sync.dma_start(out=a1s_sb[:], in_=a1s_d[:])
            a1d_sb = cp.tile([128, HID], F32, tag="a1d")
            nc.sync.dma_start(out=a1d_sb[:], in_=a1d_d[:])
            b1r_sb = cp.tile([128, HID], F32, tag="b1r")
            nc.sync.dma_start(out=b1r_sb[:], in_=b1r_d[:])
            a2s_sb = cp.tile([128, HID2], F32, tag="a2s")
            nc.sync.dma_start(out=a2s_sb[:], in_=a2s_d[:])
            a2d_sb = cp.tile([128, HID2], F32, tag="a2d")
            nc.sync.dma_start(out=a2d_sb[:], in_=a2d_d[:])
            b2r_sb = cp.tile([128, HID2], F32, tag="b2r")
            nc.sync.dma_start(out=b2r_sb[:], in_=b2r_d[:])
            Wl_sb = cp.tile([HID2 + 1, OUT], F32, tag="wl")
            nc.sync.dma_start(out=Wl_sb[:], in_=Wl_d[:])
            P_sb = cp.tile([128, PB * G], F32, tag="pp")
            nc.sync.dma_start(out=P_sb[:], in_=Pp_d[:])
            sidx_sb = cp.tile([128, S], I32, tag="sidx")
            nc.sync.dma_start(out=sidx_sb[:], in_=sidx_d[:])
            ident = cp.tile([128, 128], F32, tag="id")
            make_identity(nc, ident[:])

            T1sb = cp.tile([128, PB * R1], BF16, tag="t1")
            nc.vector.memset(T1sb[:], 0.0)
            T2sb = cp.tile([128, PB * R2], BF16, tag="t2")
            nc.vector.memset(T2sb[:], 0.0)
            sd1 = cp.tile([128, PB], F32, tag="sd1")
            sd2 = cp.tile([128, PB], F32, tag="sd2")
            ss1f = cp.tile([128, PB], F32, tag="ss1f")
            ss2f = cp.tile([128, PB], F32, tag="ss2f")

            # ---------------- phase A: h1 / scores / T1 ----------------
            with tc.tile_pool(name="xpool", bufs=3) as xp:
                for c in range(PB):
                    xc = xp.tile([128, KB * 128], F32, tag="xc")
                    xcv = xc[:].rearrange("p (k n) -> p k n", n=128)
                    nc.sync.dma_start(
                        out=xcv,
                        in_=xT_d[:, :, c * 128:(c + 1) * 128]
                        .rearrange("k p n -> p k n"))
                    ph = psA.tile([128, HID], F32, tag="ph")
                    for kb in range(KB):
                        nc.tensor.matmul(
                            ph[:],
                            lhsT=xc[:, kb * 128:(kb + 1) * 128],
                            rhs=W1_sb[:, kb * HID:(kb + 1) * HID],
                            start=(kb == 0), stop=(kb == KB - 1),
                        )
                    nc.vector.tensor_copy(
                        T1sb[:, c * R1:c * R1 + HID], ph[:])
                    tmp = wp.tile([128, HID], F32, tag="sc")
                    nc.vector.tensor_tensor(
                        out=tmp[:], in0=ph[:], in1=a1s_sb[:], op=ALU.mult)
                    nc.vector.tensor_reduce(
                        out=ss1f[:, c:c + 1], in_=tmp[:],
                        axis=mybir.AxisListType.X, op=ALU.add)
                    nc.vector.tensor_copy(
                        T1sb[:].rearrange("p (c w) -> p c w", w=R1)
                        [:, c:c + 1, HID], ss1f[:, c:c + 1])
                    tmp2 = wp.tile([128, HID], F32, tag="sc")
                    nc.vector.tensor_tensor(
                        out=tmp2[:], in0=ph[:], in1=a1d_sb[:], op=ALU.mult)
                    nc.vector.tensor_reduce(
                        out=sd1[:, c:c + 1], in_=tmp2[:],
                        axis=mybir.AxisListType.X, op=ALU.add)
                    nc.sync.dma_start(
                        out=T1shard[c * 128:(c + 1) * 128, :],
                        in_=T1sb[:, c * R1:(c + 1) * R1])
            nc.sync.dma_start(out=T1full[TR - 1:TR, :], in_=sent1_d[:])
            nc.gpsimd.collective_compute(
                "AllGather", ALU.bypass, replica_groups=groups,
                ins=[T1shard[:]], outs=[T1full[0:TR - 1, :]])

            # ---------------- phase B: GAT layer 1 ----------------
            for b in range(PB):
                L = Ls[b]
                o_t = wp.tile([128, HID], F32, tag="o1")
                den = wp.tile([128, 1], F32, tag="den")
                for si, (s0, Lc) in enumerate(subs_of(L)):
                    Gt = gp.tile([128, Lc * R1], BF16, tag="g1")
                    Gv = Gt[:].rearrange("p (l w) -> p l w", w=R1)
                    for j in range(Lc):
                        gc = gp.tile([128, R1], BF16, tag="gc")
                        nc.gpsimd.indirect_dma_start(
                            out=gc[:], out_offset=None,
                            in_=T1full[:],
                            in_offset=bass.IndirectOffsetOnAxis(
                                ap=sidx_sb[:, offs[b] + s0 + j:
                                           offs[b] + s0 + j + 1],
                                axis=0))
                        nc.vector.tensor_copy(Gv[:, j:j + 1, :], gc[:])
                    if debug_outs and b == 0 and si == 0:
                        nc.sync.dma_start(out=dbg_g[:, :Lc * R1], in_=Gt[:])
                    t_t = wp.tile([128, Lc], F32, tag="tpre")
                    nc.vector.tensor_scalar(
                        out=t_t[:], in0=Gv[:, :, HID],
                        scalar1=sd1[:, b:b + 1], scalar2=None, op0=ALU.add)
                    u_t = wp.tile([128, Lc], F32, tag="upre")
                    nc.vector.tensor_scalar(
                        out=u_t[:], in0=t_t[:], scalar1=NEG_SLOPE,
                        scalar2=None, op0=ALU.mult)
                    wl_t = wp.tile([128, Lc], F32, tag="wl1")
                    nc.vector.tensor_tensor(
                        out=wl_t[:], in0=t_t[:], in1=u_t[:], op=ALU.max)
                    wex = wp.tile([128, Lc], F32, tag="we1")
                    dsub = wp.tile([128, 1], F32, tag="dsub")
                    nc.scalar.activation(
                        wex[:], wl_t[:], AF.Exp, accum_out=dsub[:])
                    wt = wtp.tile([128, Lc * HID], F32, tag="wt")
                    nc.vector.tensor_tensor(
                        out=wt[:], in0=Gv[:, :, 0:HID],
                        in1=wex[:, :, None].to_broadcast([128, Lc, HID]),
                        op=ALU.mult)
                    if si == 0:
                        nc.vector.tensor_copy(den[:], dsub[:])
                        nc.vector.tensor_reduce(
                            out=o_t[:],
                            in_=wt[:].rearrange("p (l f) -> p f l", f=HID),
                            axis=mybir.AxisListType.X, op=ALU.add)
                    else:
                        nc.vector.tensor_tensor(
                            out=den[:], in0=den[:], in1=dsub[:], op=ALU.add)
                        o_s = wp.tile([128, HID], F32, tag="o1s")
                        nc.vector.tensor_reduce(
                            out=o_s[:],
                            in_=wt[:].rearrange("p (l f) -> p f l", f=HID),
                            axis=mybir.AxisListType.X, op=ALU.add)
                        nc.vector.tensor_tensor(
                            out=o_t[:], in0=o_t[:], in1=o_s[:], op=ALU.add)
                rec = wp.tile([128, 1], F32, tag="rec")
                nc.vector.reciprocal(rec[:], den[:])
                ob = wp.tile([128, HID], F32, tag="ob")
                nc.vector.scalar_tensor_tensor(
                    out=ob[:], in0=o_t[:], scalar=rec[:], in1=b1r_sb[:],
                    op0=ALU.mult, op1=ALU.add)
                r1 = wp.tile([128, HID], F32, tag="r1")
                nc.scalar.activation(r1[:], ob[:], AF.Relu)
                if debug_outs:
                    nc.sync.dma_start(
                        out=dbg_r1[b * 128:(b + 1) * 128, :], in_=r1[:])
                    nc.sync.dma_start(out=dbg_den[:, b:b + 1], in_=den[:])
                # transpose -> h2 = r1 @ W2, plus layer-2 scores
                pT = psB.tile([128, HID], F32, tag="tr")
                nc.tensor.transpose(pT[:], r1[:], identity=ident[:])
                r1T = wp.tile([128, HID], F32, tag="r1T")
                nc.vector.tensor_copy(r1T[:], pT[:])
                ph2 = psB.tile([128, HID2], F32, tag="tr")
                nc.tensor.matmul(ph2[:], lhsT=r1T[:], rhs=W2_sb[:],
                                 start=True, stop=True)
                nc.vector.tensor_copy(T2sb[:, b * R2:b * R2 + HID2], ph2[:])
                tmp = wp.tile([128, HID2], F32, tag="sc2")
                nc.vector.tensor_tensor(
                    out=tmp[:], in0=ph2[:], in1=a2s_sb[:], op=ALU.mult)
                nc.vector.tensor_reduce(
                    out=ss2f[:, b:b + 1], in_=tmp[:],
                    axis=mybir.AxisListType.X, op=ALU.add)
                nc.vector.tensor_copy(
                    T2sb[:].rearrange("p (c w) -> p c w", w=R2)
                    [:, b:b + 1, HID2], ss2f[:, b:b + 1])
                tmp2 = wp.tile([128, HID2], F32, tag="sc2")
                nc.vector.tensor_tensor(
                    out=tmp2[:], in0=ph2[:], in1=a2d_sb[:], op=ALU.mult)
                nc.vector.tensor_reduce(
                    out=sd2[:, b:b + 1], in_=tmp2[:],
                    axis=mybir.AxisListType.X, op=ALU.add)
                nc.sync.dma_start(
                    out=T2shard[b * 128:(b + 1) * 128, :],
                    in_=T2sb[:, b * R2:(b + 1) * R2])


            nc.sync.dma_start(out=T2full[TR - 1:TR, :], in_=sent2_d[:])
            nc.gpsimd.collective_compute(
                "AllGather", ALU.bypass, replica_groups=groups,
                ins=[T2shard[:]], outs=[T2full[0:TR - 1, :]])

            # ---------------- phase C: GAT layer 2 + pooling ----------------
            pool_ps = psP.tile([G, HID2], F32, tag="pool")
            for b in range(PB):
                L = Ls[b]
                o_t = wp.tile([128, HID2], F32, tag="o2")
                den = wp.tile([128, 1], F32, tag="den")
                for si, (s0, Lc) in enumerate(subs_of(L)):
                    Gt = gp.tile([128, Lc * R2], BF16, tag="g1")
                    Gv = Gt[:].rearrange("p (l w) -> p l w", w=R2)
                    for j in range(Lc):
                        gc = gp.tile([128, R2], BF16, tag="gc")
                        nc.gpsimd.indirect_dma_start(
                            out=gc[:], out_offset=None,
                            in_=T2full[:],
                            in_offset=bass.IndirectOffsetOnAxis(
                                ap=sidx_sb[:, offs[b] + s0 + j:
                                           offs[b] + s0 + j + 1],
                                axis=0))
                        nc.vector.tensor_copy(Gv[:, j:j + 1, :], gc[:])
                    t_t = wp.tile([128, Lc], F32, tag="tpre")
                    nc.vector.tensor_scalar(
                        out=t_t[:], in0=Gv[:, :, HID2],
                        scalar1=sd2[:, b:b + 1], scalar2=None, op0=ALU.add)
                    u_t = wp.tile([128, Lc], F32, tag="upre")
                    nc.vector.tensor_scalar(
                        out=u_t[:], in0=t_t[:], scalar1=NEG_SLOPE,
                        scalar2=None, op0=ALU.mult)
                    wl_t = wp.tile([128, Lc], F32, tag="wl1")
                    nc.vector.tensor_tensor(
                        out=wl_t[:], in0=t_t[:], in1=u_t[:], op=ALU.max)
                    wex = wp.tile([128, Lc], F32, tag="we1")
                    dsub = wp.tile([128, 1], F32, tag="dsub")
                    nc.scalar.activation(
                        wex[:], wl_t[:], AF.Exp, accum_out=dsub[:])
                    wt = wtp.tile([128, Lc * HID2], F32, tag="wt")
                    nc.vector.tensor_tensor(
                        out=wt[:], in0=Gv[:, :, 0:HID2],
                        in1=wex[:, :, None].to_broadcast([128, Lc, HID2]),
                        op=ALU.mult)
                    if si == 0:
                        nc.vector.tensor_copy(den[:], dsub[:])
                        nc.vector.tensor_reduce(
                            out=o_t[:],
                            in_=wt[:].rearrange("p (l f) -> p f l", f=HID2),
                            axis=mybir.AxisListType.X, op=ALU.add)
                    else:
                        nc.vector.tensor_tensor(
                            out=den[:], in0=den[:], in1=dsub[:], op=ALU.add)
                        o_s = wp.tile([128, HID2], F32, tag="o2s")
                        nc.vector.tensor_reduce(
                            out=o_s[:],
                            in_=wt[:].rearrange("p (l f) -> p f l", f=HID2),
                            axis=mybir.AxisListType.X, op=ALU.add)
                        nc.vector.tensor_tensor(
                            out=o_t[:], in0=o_t[:], in1=o_s[:], op=ALU.add)
                rec = wp.tile([128, 1], F32, tag="rec")
                nc.vector.reciprocal(rec[:], den[:])
                ob = wp.tile([128, HID2], F32, tag="ob2")
                nc.vector.scalar_tensor_tensor(
                    out=ob[:], in0=o_t[:], scalar=rec[:], in1=b2r_sb[:],
                    op0=ALU.mult, op1=ALU.add)
                r2 = wp.tile([128, HID2], F32, tag="r2")
                nc.scalar.activation(r2[:], ob[:], AF.Relu)
                nc.tensor.matmul(
                    pool_ps[:], lhsT=P_sb[:, b * G:(b + 1) * G], rhs=r2[:],
                    start=(b == 0), stop=(b == PB - 1))
                if debug_outs:
                    nc.sync.dma_start(
                        out=dbg_r2[b * 128:(b + 1) * 128, :], in_=r2[:])

            if debug_outs:
                nc.sync.dma_start(out=dbg_t1[:], in_=T1full[:])
                nc.sync.dma_start(out=dbg_sd1[:], in_=sd1[:])
            pooled = wp.tile([G, HID2], F32, tag="pool")
            nc.vector.tensor_copy(pooled[:], pool_ps[:])
            nc.sync.dma_start(out=poolin[:], in_=pooled[:])
            nc.gpsimd.collective_compute(
                "AllReduce", ALU.add, replica_groups=groups,
                ins=[poolin[:]], outs=[poolout[:]])
            pooled_r = wp.tile([G, HID2], F32, tag="poolr")
            nc.sync.dma_start(out=pooled_r[:], in_=poolout[:])
            pTf = psB.tile([HID2, G], F32, tag="tr")
            nc.tensor.transpose(pTf[:], pooled_r[:], identity=ident[:G, :G])
            fin = wp.tile([HID2 + 1, G], F32, tag="fin")
            nc.vector.tensor_copy(fin[:HID2, :], pTf[:])
            nc.vector.memset(fin[HID2:HID2 + 1, :], 1.0)
            out_ps = psB.tile([G, OUT], F32, tag="tr")
            nc.tensor.matmul(out_ps[:], lhsT=fin[:], rhs=Wl_sb[:],
                             start=True, stop=True)
            out_sb = wp.tile([G, OUT], F32, tag="outsb")
            nc.vector.tensor_copy(out_sb[:], out_ps[:])
            nc.sync.dma_start(out=out_d[:], in_=out_sb[:])

    nc.compile()
    return nc


# ----------------------------------------------------------------------------
# Entry point
# ----------------------------------------------------------------------------

LAST_RESULTS = None


def kernel(**inputs):
    global LAST_RESULTS
    cfg = full_cfg()
    in_maps, meta = preprocess(cfg=cfg, **inputs)
    nc = build_program(cfg, meta)
    res = run_bass_kernel_spmd(nc, in_maps, core_ids=list(range(cfg["NCORES"])))
    LAST_RESULTS = res
    return np.asarray(res.results[0]["out"], np.float32)



# revision 4
# speedup vs baseline: 1.7402x; 1.7402x over previous
"""Trainium2 Bass kernel for a 2-layer GAT + global mean pool + linear head.

Math (matches PyG GATConv, eval mode, single head, add_self_loops=True):
  h   = x @ W
  e_k = lrelu(ss[src_k] + sd[dst_k]),  ss = h@a_src, sd = h@a_dst
  alpha = softmax over incoming edges of each dst (self-loop included)
  out[d] = sum_k alpha_k h[src_k] + b
Two GAT layers (512->128, 128->64) with ReLU, then per-graph mean pool
over `batch` and a final [64,2] linear.

Strategy (8 NeuronCores, full inputs in / full output out):
  * Destination nodes sharded across cores (2500/core), sources arbitrary.
  * Weight matrices extended with W@a_src / W@a_dst columns so one dense
    matmul yields [h | ss | sd] per node.
  * Each core publishes a 256-byte gather-table row per node and
    AllGathers the table:  layer 1 row = [h[0:126] bf16 | h[126:128]
    fp8e4m3 | ss bf16],  layer 2 row = [h2 bf16 | ss2 bf16 | 0-pad].
    256B rows hit dma_gather's fastest descriptor size.
  * Edges grouped per destination into fixed "slots" (sentinel-padded)
    destination-per-partition.  SWDGE dma_gather fetches slot source
    rows in <=1024-descriptor chunks (~79ns/descriptor measured; the
    vector-indirect InstDMACopy path is ~10x slower per descriptor).
  * Attention: per-block score ops (ACT bias-add + Exp with accum_out
    producing the softmax denominator), then per slot column a
    scale-by-exp(e) (DVE/ACT) and an identity-matmul on PE accumulating
    the weighted rows in PSUM.
  * Per-graph pooling one-hots (1/count folded in) are host-built bf16;
    partial pooled features are AllReduced, final linear on every core.

All graph-structure preprocessing (degree sort, slot layout, index
remapping, 16-partition index wrapping) is host-side numpy on the
kernel inputs; the device only sees dense arrays.
"""

import math
import numpy as np

import concourse.bass as bass
import concourse.bacc as bacc
import concourse.mybir as mybir
from concourse.tile import TileContext
from concourse.masks import make_identity
from concourse.bass_utils import run_bass_kernel_spmd

F32 = mybir.dt.float32
BF16 = mybir.dt.bfloat16
F8E4 = mybir.dt.float8e4
I16 = mybir.dt.int16
AF = mybir.ActivationFunctionType
ALU = mybir.AluOpType

NEG_SLOPE = 0.2
SENT_SS = -60.0  # sentinel row score: exp(lrelu(-60+sd)) ~ e^-11 -> harmless
ACT_EVERY = 3    # every ACT_EVERY-th slot column's scale runs on ScalarE
GCHUNK = 8       # slot columns per dma_gather (8*128 = 1024 descriptors)


def full_cfg():
    return dict(N=20000, IND=512, HID=128, HID2=64, OUT=2, G=16, NCORES=8)


# ----------------------------------------------------------------------------
# Host-side preprocessing
# ----------------------------------------------------------------------------

def preprocess(x, edge_index, batch, W1, a1_src, a1_dst, b1,
               W2, a2_src, a2_dst, b2, Wl, bl, cfg):
    N, IND, HID, HID2, OUT, G, NC = (cfg[k] for k in
                                     ("N", "IND", "HID", "HID2", "OUT", "G",
                                      "NCORES"))
    PC = math.ceil(N / NC)            # real dests per core
    PB = math.ceil(PC / 128)          # dest blocks per core
    PCP = PB * 128                    # padded dests per core
    TR = NC * PCP + 1                 # table rows (+1 sentinel)
    SENT = TR - 1
    KB = IND // 128
    R = 128                           # table row: 128 bf16 elems = 256B

    import ml_dtypes
    BF = ml_dtypes.bfloat16

    x = np.asarray(x, np.float32)
    batch = np.asarray(batch, np.int64)
    src = np.asarray(edge_index[0], np.int64)
    dst = np.asarray(edge_index[1], np.int64)
    # self loops
    loop = np.arange(N, dtype=np.int64)
    src = np.concatenate([src, loop])
    dst = np.concatenate([dst, loop])

    counts = np.bincount(batch, minlength=G).astype(np.float64)

    # per-core degree-sorted permutations and global row ids
    row_of = np.empty(N, np.int64)       # global node -> table row
    orders = []
    degs_sorted = np.zeros((NC, PCP), np.int64)
    for k in range(NC):
        lo, hi = k * PC, min((k + 1) * PC, N)
        nk = hi - lo
        mask = (dst >= lo) & (dst < hi)
        deg = np.bincount(dst[mask] - lo, minlength=nk)
        order = np.argsort(-deg, kind="stable")        # local rank -> local id
        inv = np.empty(nk, np.int64)
        inv[order] = np.arange(nk)
        row_of[lo:hi] = k * PCP + inv
        orders.append(order)
        degs_sorted[k, :nk] = deg[order]

    # global per-block slot counts (identical program on every core)
    Ls = []
    for b in range(PB):
        Lb = int(degs_sorted[:, b * 128:(b + 1) * 128].max())
        Ls.append(max(Lb, 1))
    S = int(np.sum(Ls))
    offs = np.concatenate([[0], np.cumsum(Ls)]).astype(np.int64)

    # shared (replicated) weight uploads: extended with score columns
    W1e = np.concatenate([
        np.asarray(W1, np.float32),
        (np.asarray(W1, np.float32) @ np.asarray(a1_src, np.float32))[:, None],
        (np.asarray(W1, np.float32) @ np.asarray(a1_dst, np.float32))[:, None],
    ], axis=1)                                   # [512, 130] = [h|ss|sd]
    W1u = np.ascontiguousarray(W1e.reshape(KB, 128, HID + 2)).astype(BF)
    W2e = np.concatenate([
        np.asarray(W2, np.float32),
        (np.asarray(W2, np.float32) @ np.asarray(a2_src, np.float32))[:, None],
        (np.asarray(W2, np.float32) @ np.asarray(a2_dst, np.float32))[:, None],
    ], axis=1).astype(BF)                        # [128, 66]
    b1r = np.tile(np.asarray(b1, np.float32)[None, :], (128, 1))
    b2r = np.tile(np.asarray(b2, np.float32)[None, :], (128, 1))
    WlBl = np.concatenate([np.asarray(Wl, np.float32),
                           np.asarray(bl, np.float32)[None, :]], axis=0)
    sent = np.zeros((1, R), BF)
    sent[0, R - 1] = SENT_SS      # layer-1 sentinel: ss at elem 127
    sent2 = np.zeros((1, R), BF)
    sent2[0, HID2] = SENT_SS      # layer-2 sentinel: ss2 at elem 64

    in_maps = []
    for k in range(NC):
        lo, hi = k * PC, min((k + 1) * PC, N)
        nk = hi - lo
        order = orders[k]

        # xT: [128, PB, KB*128] partition = feature-within-chunk, so
        # xT[p, c, kb*128+m] = xs[c*128+m, kb*128+p]  (1KB contiguous/desc)
        xs = np.zeros((PCP, IND), np.float32)
        xs[:nk] = x[lo:hi][order]
        xT = np.ascontiguousarray(
            xs.reshape(PB, 128, KB, 128).transpose(3, 0, 2, 1)
            .reshape(128, PB, KB * 128)).astype(BF)

        # slot indices [128, S] -> table rows, sentinel padded
        sidx = np.full((128, S), SENT, np.int64)
        mask = (dst >= lo) & (dst < hi)
        es, ed = src[mask], dst[mask] - lo
        o = np.argsort(ed, kind="stable")
        es, ed = es[o], ed[o]
        deg = np.bincount(ed, minlength=nk)
        start = np.concatenate([[0], np.cumsum(deg)[:-1]])
        j = np.arange(len(ed)) - start[ed]            # slot within dest
        inv = np.empty(nk, np.int64)
        inv[order] = np.arange(nk)
        r = inv[ed]                                   # dest rank
        bb, pp = r // 128, r % 128
        col = offs[bb] + j
        sidx[pp, col] = row_of[es]

        # dma_gather wrapped indices: per chunk of <=GCHUNK slot columns,
        # flat[i] lands at out partition i%128, column i//128;
        # idxs[p, c] = flat[c*16 + p%16], replicated over the 8 q7 cores.
        wcols = []
        for b in range(PB):
            L = Ls[b]
            for c0 in range(0, L, GCHUNK):
                CB = min(GCHUNK, L - c0)
                flat = sidx[:, offs[b] + c0: offs[b] + c0 + CB].T.reshape(-1)
                wcols.append(np.tile(flat.reshape(-1, 16).T, (8, 1)))
        widx = np.concatenate(wcols, axis=1).astype(np.int16)  # [128, S*8]

        # pooling one-hot with 1/count folded, zero rows for pad dests
        P = np.zeros((128, PB * G), np.float32)
        bg = batch[lo:hi][order]                      # graph id per rank
        rr = np.arange(nk)
        P[rr % 128, (rr // 128) * G + bg] = 1.0 / np.maximum(counts[bg], 1.0)

        in_maps.append(dict(
            xT=xT, W1u=W1u, W2u=W2e, b1r=b1r, b2r=b2r,
            WlBl=WlBl.astype(np.float32),
            Pp=P.astype(BF), widx=widx, sent1=sent, sent2=sent2,
        ))

    meta = dict(PC=PC, PB=PB, PCP=PCP, TR=TR, R=R, KB=KB, S=S,
                Ls=Ls, offs=offs)
    return in_maps, meta


# ----------------------------------------------------------------------------
# Device program
# ----------------------------------------------------------------------------

def build_program(cfg, meta, sim_mode=False):
    N, IND, HID, HID2, OUT, G, NC = (cfg[k] for k in
                                     ("N", "IND", "HID", "HID2", "OUT",
                                      "G", "NCORES"))
    PB, PCP, TR, R, KB, S = (meta[k] for k in
                             ("PB", "PCP", "TR", "R", "KB", "S"))
    Ls, offs = meta["Ls"], meta["offs"]
    V1 = HID + 2        # phase-A psum width  [h | ss | sd]
    V2 = HID2 + 2

    ndev = 1 if sim_mode else NC
    nc = bacc.Bacc("TRN2", target_bir_lowering=False, debug=False,
                   num_devices=ndev)

    xT_d = nc.declare_dram_parameter("xT", [128, PB, KB * 128], BF16, False)
    W1_d = nc.declare_dram_parameter("W1u", [KB, 128, V1], BF16, False)
    W2_d = nc.declare_dram_parameter("W2u", [HID, V2], BF16, False)
    b1r_d = nc.declare_dram_parameter("b1r", [128, HID], F32, False)
    b2r_d = nc.declare_dram_parameter("b2r", [128, HID2], F32, False)
    Wl_d = nc.declare_dram_parameter("WlBl", [HID2 + 1, OUT], F32, False)
    Pp_d = nc.declare_dram_parameter("Pp", [128, PB * G], BF16, False)
    widx_d = nc.declare_dram_parameter("widx", [128, S * GCHUNK], I16, False)
    sent1_d = nc.declare_dram_parameter("sent1", [1, R], BF16, False)
    sent2_d = nc.declare_dram_parameter("sent2", [1, R], BF16, False)
    out_d = nc.declare_dram_parameter("out", [G, OUT], F32, True)

    shared = dict(addr_space="Shared") if (not sim_mode and NC > 4) else {}
    T1shard = nc.dram_tensor("T1shard", [PCP, R], BF16)
    T1full = nc.dram_tensor("T1full", [TR, R], BF16, **shared)
    T2shard = nc.dram_tensor("T2shard", [PCP, R], BF16)
    T2full = nc.dram_tensor("T2full", [TR, R], BF16, **shared)
    poolin = nc.dram_tensor("poolin", [G, HID2], F32)
    poolout = nc.dram_tensor("poolout", [G, HID2], F32, **shared)

    groups = [list(range(NC))]

    def gather_block(Gt, Tfull, widx_sb, b):
        L = Ls[b]
        c0 = 0
        while c0 < L:
            CB = min(GCHUNK, L - c0)
            Gvc = Gt[:, c0 * R:(c0 + CB) * R].rearrange(
                "p (c e) -> p c e", e=R)
            w0 = (offs[b] + c0) * GCHUNK
            nc.gpsimd.dma_gather(
                out_ap=Gvc, in_ap=Tfull[:],
                idxs_ap=widx_sb[:, w0:w0 + CB * GCHUNK],
                num_idxs=CB * 128, num_idxs_reg=CB * 128, elem_size=R)
            c0 += CB

    with TileContext(nc) as tc:
        with (
            tc.tile_pool(name="const", bufs=1) as cp,
            tc.tile_pool(name="work", bufs=3) as wp,
            tc.tile_pool(name="gath", bufs=2) as gp,
            tc.tile_pool(name="rsp", bufs=4) as rsp,
            tc.tile_pool(name="xpool", bufs=3) as xp,
            tc.tile_pool(name="psA", bufs=2, space="PSUM") as psA,
            tc.tile_pool(name="psB", bufs=2, space="PSUM") as psB,
            tc.tile_pool(name="psP", bufs=1, space="PSUM") as psP,
        ):
            # ---------------- constants to SBUF ----------------
            W1_sb = cp.tile([128, KB * V1], BF16, tag="w1")
            W1v = W1_sb[:].rearrange("p (k h) -> p k h", h=V1)
            nc.sync.dma_start(out=W1v, in_=W1_d[:].rearrange("k p h -> p k h"))
            W2_sb = cp.tile([HID, V2], BF16, tag="w2")
            nc.sync.dma_start(out=W2_sb[:], in_=W2_d[:])
            b1r_sb = cp.tile([128, HID], F32, tag="b1r")
            nc.sync.dma_start(out=b1r_sb[:], in_=b1r_d[:])
            b2r_sb = cp.tile([128, HID2], F32, tag="b2r")
            nc.sync.dma_start(out=b2r_sb[:], in_=b2r_d[:])
            Wl_sb = cp.tile([HID2 + 1, OUT], F32, tag="wl")
            nc.sync.dma_start(out=Wl_sb[:], in_=Wl_d[:])
            P_sb = cp.tile([128, PB * G], BF16, tag="pp")
            nc.sync.dma_start(out=P_sb[:], in_=Pp_d[:])
            widx_sb = cp.tile([128, S * GCHUNK], I16, tag="widx")
            nc.sync.dma_start(out=widx_sb[:], in_=widx_d[:])
            identB = cp.tile([128, 128], BF16, tag="idb")
            make_identity(nc, identB[:])
            identF = cp.tile([G, G], F32, tag="idf")
            make_identity(nc, identF[:])

            T1sb = cp.tile([128, PB * R], BF16, tag="t1")
            T1sb8 = T1sb[:].bitcast(F8E4)        # [128, PB*256] fp8 view
            T2sb = cp.tile([128, PB * R], BF16, tag="t2")
            nc.vector.memset(T2sb[:], 0.0)
            sd1 = cp.tile([128, PB], F32, tag="sd1")
            sd2 = cp.tile([128, PB], F32, tag="sd2")

            # ------- phase A: [h|ss|sd] per node, build T1 rows -------
            for c in range(PB):
                xc = xp.tile([128, KB * 128], BF16, tag="xc")
                nc.sync.dma_start(out=xc[:], in_=xT_d[:, c, :])
                ph = psA.tile([128, V1], F32, tag="acc")
                for kb in range(KB):
                    nc.tensor.matmul(
                        ph[:],
                        lhsT=xc[:, kb * 128:(kb + 1) * 128],
                        rhs=W1_sb[:, kb * V1:(kb + 1) * V1],
                        start=(kb == 0), stop=(kb == KB - 1),
                    )
                nc.vector.tensor_copy(
                    T1sb[:, c * R:c * R + 126], ph[:, 0:126])
                nc.vector.tensor_copy(
                    T1sb8[:, c * 256 + 252:c * 256 + 254], ph[:, 126:128])
                nc.vector.tensor_copy(
                    T1sb[:, c * R + 127:c * R + 128], ph[:, HID:HID + 1])
                nc.vector.tensor_copy(sd1[:, c:c + 1], ph[:, HID + 1:HID + 2])
                nc.sync.dma_start(
                    out=T1shard[c * 128:(c + 1) * 128, :],
                    in_=T1sb[:, c * R:(c + 1) * R])

            nc.sync.dma_start(out=T1full[TR - 1:TR, :], in_=sent1_d[:])
            if sim_mode:
                nc.sync.dma_start(out=T1full[0:PCP, :], in_=T1shard[:])
            else:
                nc.gpsimd.collective_compute(
                    "AllGather", ALU.bypass, replica_groups=groups,
                    ins=[T1shard[:]], outs=[T1full[0:TR - 1, :]])

            # ---------------- phase B: GAT layer 1 + T2 build ----------------
            for b in range(PB):
                L = Ls[b]
                Gt = gp.tile([128, L * R], BF16, tag="g1")
                Gv = Gt[:].rearrange("p (l w) -> p l w", w=R)
                Gt8 = Gt[:].bitcast(F8E4)            # [128, L*256]
                gather_block(Gt, T1full, widx_sb, b)
                # scores: e = lrelu(ss_src + sd_dst); w = exp(e); den = sum w
                t_t = wp.tile([128, L], F32, tag="tpre")
                nc.scalar.activation(t_t[:], Gv[:, :, R - 1], AF.Identity,
                                     bias=sd1[:, b:b + 1])
                u_t = wp.tile([128, L], F32, tag="upre")
                nc.vector.tensor_scalar(
                    out=u_t[:], in0=t_t[:], scalar1=NEG_SLOPE,
                    scalar2=None, op0=ALU.mult)
                wl_t = wp.tile([128, L], F32, tag="wl1")
                nc.vector.tensor_tensor(
                    out=wl_t[:], in0=t_t[:], in1=u_t[:], op=ALU.max)
                wex = wp.tile([128, L], F32, tag="we1")
                den = wp.tile([128, 1], F32, tag="den")
                nc.scalar.activation(wex[:], wl_t[:], AF.Exp,
                                     accum_out=den[:])
                rec = wp.tile([128, 1], F32, tag="rec")
                nc.vector.reciprocal(rec[:], den[:])
                # weighted accumulate via identity matmul
                acc = psA.tile([128, HID], F32, tag="acc")
                for j in range(L):
                    rs = rsp.tile([128, HID], BF16, tag="rs")
                    if j % ACT_EVERY == ACT_EVERY - 1:
                        nc.scalar.activation(rs[:, 0:126], Gv[:, j, 0:126],
                                             AF.Copy, scale=wex[:, j:j + 1])
                    else:
                        nc.vector.tensor_scalar(
                            out=rs[:, 0:126], in0=Gv[:, j, 0:126],
                            scalar1=wex[:, j:j + 1], scalar2=None,
                            op0=ALU.mult)
                    nc.vector.tensor_scalar(
                        out=rs[:, 126:128],
                        in0=Gt8[:, j * 256 + 252:j * 256 + 254],
                        scalar1=wex[:, j:j + 1], scalar2=None, op0=ALU.mult)
                    nc.tensor.matmul(acc[:], lhsT=identB[:], rhs=rs[:],
                                     start=(j == 0), stop=(j == L - 1))
                ob = wp.tile([128, HID], F32, tag="ob")
                nc.vector.scalar_tensor_tensor(
                    out=ob[:], in0=acc[:], scalar=rec[:],
                    in1=b1r_sb[:], op0=ALU.mult, op1=ALU.add)
                r1b = wp.tile([128, HID], BF16, tag="r1b")
                nc.scalar.activation(r1b[:], ob[:], AF.Relu)
                # h2 = relu(h1) @ W2e -> [h2 | ss2 | sd2]
                pT = psB.tile([128, HID], BF16, tag="tr")
                nc.tensor.transpose(pT[:], r1b[:], identity=identB[:])
                r1T = wp.tile([128, HID], BF16, tag="r1T")
                nc.vector.tensor_copy(r1T[:], pT[:])
                ph2 = psB.tile([128, V2], F32, tag="tr")
                nc.tensor.matmul(ph2[:], lhsT=r1T[:], rhs=W2_sb[:],
                                 start=True, stop=True)
                nc.vector.tensor_copy(
                    T2sb[:, b * R:b * R + HID2 + 1], ph2[:, 0:HID2 + 1])
                nc.vector.tensor_copy(sd2[:, b:b + 1],
                                      ph2[:, HID2 + 1:HID2 + 2])
                nc.sync.dma_start(
                    out=T2shard[b * 128:(b + 1) * 128, :],
                    in_=T2sb[:, b * R:(b + 1) * R])

            nc.sync.dma_start(out=T2full[TR - 1:TR, :], in_=sent2_d[:])
            if sim_mode:
                nc.sync.dma_start(out=T2full[0:PCP, :], in_=T2shard[:])
            else:
                nc.gpsimd.collective_compute(
                    "AllGather", ALU.bypass, replica_groups=groups,
                    ins=[T2shard[:]], outs=[T2full[0:TR - 1, :]])

            # ---------------- phase C: GAT layer 2 + pooling ----------------
            pool_ps = psP.tile([G, HID2], F32, tag="pool")
            for b in range(PB):
                L = Ls[b]
                Gt = gp.tile([128, L * R], BF16, tag="g1")
                Gv = Gt[:].rearrange("p (l w) -> p l w", w=R)
                gather_block(Gt, T2full, widx_sb, b)
                t_t = wp.tile([128, L], F32, tag="tpre")
                nc.scalar.activation(t_t[:], Gv[:, :, HID2], AF.Identity,
                                     bias=sd2[:, b:b + 1])
                u_t = wp.tile([128, L], F32, tag="upre")
                nc.vector.tensor_scalar(
                    out=u_t[:], in0=t_t[:], scalar1=NEG_SLOPE,
                    scalar2=None, op0=ALU.mult)
                wl_t = wp.tile([128, L], F32, tag="wl1")
                nc.vector.tensor_tensor(
                    out=wl_t[:], in0=t_t[:], in1=u_t[:], op=ALU.max)
                wex = wp.tile([128, L], F32, tag="we1")
                den = wp.tile([128, 1], F32, tag="den")
                nc.scalar.activation(wex[:], wl_t[:], AF.Exp,
                                     accum_out=den[:])
                rec = wp.tile([128, 1], F32, tag="rec")
                nc.vector.reciprocal(rec[:], den[:])
                acc = psA.tile([128, HID2], F32, tag="acc")
                for j in range(L):
                    rs = rsp.tile([128, HID2], BF16, tag="rs")
                    if j % ACT_EVERY == ACT_EVERY - 1:
                        nc.scalar.activation(rs[:], Gv[:, j, 0:HID2],
                                             AF.Copy, scale=wex[:, j:j + 1])
                    else:
                        nc.vector.tensor_scalar(
                            out=rs[:], in0=Gv[:, j, 0:HID2],
                            scalar1=wex[:, j:j + 1], scalar2=None,
                            op0=ALU.mult)
                    nc.tensor.matmul(acc[:], lhsT=identB[:], rhs=rs[:],
                                     start=(j == 0), stop=(j == L - 1))
                ob2 = wp.tile([128, HID2], F32, tag="ob2")
                nc.vector.scalar_tensor_tensor(
                    out=ob2[:], in0=acc[:], scalar=rec[:],
                    in1=b2r_sb[:], op0=ALU.mult, op1=ALU.add)
                r2b = wp.tile([128, HID2], BF16, tag="r2b")
                nc.scalar.activation(r2b[:], ob2[:], AF.Relu)
                nc.tensor.matmul(
                    pool_ps[:], lhsT=P_sb[:, b * G:(b + 1) * G], rhs=r2b[:],
                    start=(b == 0), stop=(b == PB - 1))

            pooled = wp.tile([G, HID2], F32, tag="pool")
            nc.vector.tensor_copy(pooled[:], pool_ps[:])
            nc.sync.dma_start(out=poolin[:], in_=pooled[:])
            if sim_mode:
                nc.sync.dma_start(out=poolout[:], in_=poolin[:])
            else:
                nc.gpsimd.collective_compute(
                    "AllReduce", ALU.add, replica_groups=groups,
                    ins=[poolin[:]], outs=[poolout[:]])
            pooled_r = wp.tile([G, HID2], F32, tag="poolr")
            nc.sync.dma_start(out=pooled_r[:], in_=poolout[:])
            pTf = psB.tile([HID2, G], F32, tag="tr")
            nc.tensor.transpose(pTf[:], pooled_r[:], identity=identF[:])
            fin = wp.tile([HID2 + 1, G], F32, tag="fin")
            nc.vector.tensor_copy(fin[:HID2, :], pTf[:])
            nc.vector.memset(fin[HID2:HID2 + 1, :], 1.0)
            out_ps = psB.tile([G, OUT], F32, tag="tr")
            nc.tensor.matmul(out_ps[:], lhsT=fin[:], rhs=Wl_sb[:],
                             start=True, stop=True)
            out_sb = wp.tile([G, OUT], F32, tag="outsb")
            nc.vector.tensor_copy(out_sb[:], out_ps[:])
            nc.sync.dma_start(out=out_d[:], in_=out_sb[:])

    nc.compile()
    return nc


# ----------------------------------------------------------------------------
# Entry point
# ----------------------------------------------------------------------------

LAST_RESULTS = None


def kernel(**inputs):
    global LAST_RESULTS
    cfg = full_cfg()
    in_maps, meta = preprocess(cfg=cfg, **inputs)
    nc = build_program(cfg, meta)
    res = run_bass_kernel_spmd(nc, in_maps, core_ids=list(range(cfg["NCORES"])))
    LAST_RESULTS = res
    return np.asarray(res.results[0]["out"], np.float32)


# revision 15
# speedup vs baseline: 167.4341x; 96.2128x over previous
"""Trainium2 Bass kernel for a 2-layer GAT + global mean pool + linear head.

Math (matches PyG GATConv, eval mode, single head, add_self_loops=True):
  h   = x @ W
  e_k = lrelu(ss[src_k] + sd[dst_k]),  ss = h@a_src, sd = h@a_dst
  alpha = softmax over incoming edges of each dst (self-loop included)
  out[d] = sum_k alpha_k h[src_k] + b
Two GAT layers (512->128, 128->64) with ReLU, then per-graph mean pool
over `batch` and a final [64,2] linear.

Strategy (8 NeuronCores, full inputs in / full output out):
  * Destination nodes sharded across cores (2500/core), sources arbitrary.
  * Weight matrices extended with W@a_src / W@a_dst columns so one dense
    matmul yields [h | ss | sd] per node.
  * Each core publishes a 256-byte gather-table row per node and
    AllGathers the table:
      layer-1 row = [h[0:126] bf16 | ss bf16 | h[126:128] fp8e4m3]
      layer-2 row = [h2 bf16 | ss2 bf16 | sd2 bf16 | 0-pad]
    256B rows hit dma_gather's fastest descriptor size.
  * Edges grouped per destination into fixed "slots" (sentinel-padded)
    destination-per-partition.  SWDGE dma_gather fetches slot source
    rows in 1024-descriptor chunks (the hardware per-instruction cap).
  * Per 128-destination block the whole attention layer is a handful of
    wide fused ops: ACT bias-add, one fused lrelu (scalar_tensor_tensor
    max(0.2t, t)), ACT Exp with accum_out as the softmax denominator,
    one broadcast-multiply, one strided tensor_reduce.  This matters
    because the execution environment has a large per-instruction
    dispatch overhead, so wall time ~ instruction count.
  * Per-graph pooling one-hots (1/count folded in) are host-built bf16;
    partial pooled features are AllReduced, final linear on every core.

All graph-structure preprocessing (degree sort, slot layout, index
remapping, 16-partition index wrapping) is host-side numpy on the
kernel inputs; the device only sees dense arrays.
"""

import math
import numpy as np

import concourse.bass as bass
import concourse.bacc as bacc
import concourse.mybir as mybir
from concourse.tile import TileContext
from concourse.masks import make_identity
from concourse.bass_utils import run_bass_kernel_spmd

F32 = mybir.dt.float32
BF16 = mybir.dt.bfloat16
F8E4 = mybir.dt.float8e4
I16 = mybir.dt.int16
AF = mybir.ActivationFunctionType
ALU = mybir.AluOpType

NEG_SLOPE = 0.2
SENT_SS = -60.0  # sentinel row score: exp(lrelu(-60+sd)) ~ e^-11 -> harmless
GCHUNK = 8       # slot columns per dma_gather (8*128 = 1024 descriptors)


def full_cfg():
    return dict(N=20000, IND=512, HID=128, HID2=64, OUT=2, G=16, NCORES=8)


# ----------------------------------------------------------------------------
# Host-side preprocessing
# ----------------------------------------------------------------------------

def preprocess(x, edge_index, batch, W1, a1_src, a1_dst, b1,
               W2, a2_src, a2_dst, b2, Wl, bl, cfg):
    N, IND, HID, HID2, OUT, G, NC = (cfg[k] for k in
                                     ("N", "IND", "HID", "HID2", "OUT", "G",
                                      "NCORES"))
    PC = math.ceil(N / NC)            # real dests per core
    PB = math.ceil(PC / 128)          # dest blocks per core
    PCP = PB * 128                    # padded dests per core
    TR = NC * PCP + 1                 # table rows (+1 sentinel)
    SENT = TR - 1
    KB = IND // 128
    R = 128                           # table row: 128 bf16 elems = 256B

    import ml_dtypes
    BF = ml_dtypes.bfloat16

    x = np.asarray(x, np.float32)
    batch = np.asarray(batch, np.int64)
    src = np.asarray(edge_index[0], np.int64)
    dst = np.asarray(edge_index[1], np.int64)
    # self loops
    loop = np.arange(N, dtype=np.int64)
    src = np.concatenate([src, loop])
    dst = np.concatenate([dst, loop])

    counts = np.bincount(batch, minlength=G).astype(np.float64)

    # per-core degree-sorted permutations and global row ids
    row_of = np.empty(N, np.int64)       # global node -> table row
    orders = []
    degs_sorted = np.zeros((NC, PCP), np.int64)
    for k in range(NC):
        lo, hi = k * PC, min((k + 1) * PC, N)
        nk = hi - lo
        mask = (dst >= lo) & (dst < hi)
        deg = np.bincount(dst[mask] - lo, minlength=nk)
        order = np.argsort(-deg, kind="stable")        # local rank -> local id
        inv = np.empty(nk, np.int64)
        inv[order] = np.arange(nk)
        row_of[lo:hi] = k * PCP + inv
        orders.append(order)
        degs_sorted[k, :nk] = deg[order]

    # global per-block slot counts (identical program on every core),
    # padded to be equal within each pair of consecutive blocks so both
    # blocks of a pair can share one set of fused score/aggregation ops
    Ls = []
    for b in range(PB):
        Lb = int(degs_sorted[:, b * 128:(b + 1) * 128].max())
        Ls.append(max(Lb, 1))
    for i in range(0, PB - 1, 2):
        Lp = max(Ls[i], Ls[i + 1])
        Ls[i] = Ls[i + 1] = Lp
    S = int(np.sum(Ls))
    offs = np.concatenate([[0], np.cumsum(Ls)]).astype(np.int64)

    # extended weights; layer-1 psum order [h0:126 | ss | h126 | h127 | sd]
    # so the table row [h0:126 | ss] is one contiguous psum copy.
    W1f = np.asarray(W1, np.float32)
    W1e = np.concatenate([
        W1f[:, 0:126],
        (W1f @ np.asarray(a1_src, np.float32))[:, None],
        W1f[:, 126:128],
        (W1f @ np.asarray(a1_dst, np.float32))[:, None],
    ], axis=1)                                   # [512, 130]
    W1u = np.ascontiguousarray(W1e.reshape(KB, 128, HID + 2)).astype(BF)
    W2f = np.asarray(W2, np.float32)
    W2e = np.concatenate([
        W2f,
        (W2f @ np.asarray(a2_src, np.float32))[:, None],
        (W2f @ np.asarray(a2_dst, np.float32))[:, None],
    ], axis=1).astype(BF)                        # [128, 66] = [h2|ss2|sd2]
    b1r = np.tile(np.asarray(b1, np.float32)[None, :], (128, 1))
    b2r = np.tile(np.asarray(b2, np.float32)[None, :], (128, 1))
    WlBl = np.concatenate([np.asarray(Wl, np.float32),
                           np.asarray(bl, np.float32)[None, :]], axis=0)
    sent = np.zeros((1, R), BF)
    sent[0, 126] = SENT_SS        # layer-1 sentinel: ss at elem 126
    sent2 = np.zeros((1, R), BF)
    sent2[0, HID2] = SENT_SS      # layer-2 sentinel: ss2 at elem 64

    in_maps = []
    for k in range(NC):
        lo, hi = k * PC, min((k + 1) * PC, N)
        nk = hi - lo
        order = orders[k]

        # xT: [128, PB, KB*128] partition = feature-within-chunk, so
        # xT[p, c, kb*128+m] = xs[c*128+m, kb*128+p]  (1KB contiguous/desc)
        xs = np.zeros((PCP, IND), np.float32)
        xs[:nk] = x[lo:hi][order]
        xT = np.ascontiguousarray(
            xs.reshape(PB, 128, KB, 128).transpose(3, 0, 2, 1)
            .reshape(128, PB, KB * 128)).astype(BF)

        # slot indices [128, S] -> table rows, sentinel padded
        sidx = np.full((128, S), SENT, np.int64)
        mask = (dst >= lo) & (dst < hi)
        es, ed = src[mask], dst[mask] - lo
        o = np.argsort(ed, kind="stable")
        es, ed = es[o], ed[o]
        deg = np.bincount(ed, minlength=nk)
        start = np.concatenate([[0], np.cumsum(deg)[:-1]])
        j = np.arange(len(ed)) - start[ed]            # slot within dest
        inv = np.empty(nk, np.int64)
        inv[order] = np.arange(nk)
        r = inv[ed]                                   # dest rank
        bb, pp = r // 128, r % 128
        col = offs[bb] + j
        sidx[pp, col] = row_of[es]

        # dma_gather wrapped indices: per chunk of <=GCHUNK slot columns,
        # flat[i] lands at out partition i%128, column i//128;
        # idxs[p, c] = flat[c*16 + p%16], replicated over the 8 q7 cores.
        wcols = []
        for b in range(PB):
            L = Ls[b]
            for c0 in range(0, L, GCHUNK):
                CB = min(GCHUNK, L - c0)
                flat = sidx[:, offs[b] + c0: offs[b] + c0 + CB].T.reshape(-1)
                wcols.append(np.tile(flat.reshape(-1, 16).T, (8, 1)))
        widx = np.concatenate(wcols, axis=1).astype(np.int16)  # [128, S*8]

        # pooling one-hot with 1/count folded, zero rows for pad dests
        P = np.zeros((128, PB * G), np.float32)
        bg = batch[lo:hi][order]                      # graph id per rank
        rr = np.arange(nk)
        P[rr % 128, (rr // 128) * G + bg] = 1.0 / np.maximum(counts[bg], 1.0)

        in_maps.append(dict(
            xT=xT, W1u=W1u, W2u=W2e, b1r=b1r, b2r=b2r,
            WlBl=WlBl.astype(np.float32),
            Pp=P.astype(BF), widx=widx, sent1=sent, sent2=sent2,
        ))

    meta = dict(PC=PC, PB=PB, PCP=PCP, TR=TR, R=R, KB=KB, S=S,
                Ls=Ls, offs=offs)
    return in_maps, meta


# ----------------------------------------------------------------------------
# Device program
# ----------------------------------------------------------------------------

def build_program(cfg, meta, sim_mode=False, reps=1, rep_colls=True, phase_reps=None):
    N, IND, HID, HID2, OUT, G, NC = (cfg[k] for k in
                                     ("N", "IND", "HID", "HID2", "OUT",
                                      "G", "NCORES"))
    PB, PCP, TR, R, KB, S = (meta[k] for k in
                             ("PB", "PCP", "TR", "R", "KB", "S"))
    Ls, offs = meta["Ls"], meta["offs"]
    V1 = HID + 2        # phase-A psum width  [h0:126 | ss | h126:128 | sd]
    V2 = HID2 + 2

    ndev = 1 if sim_mode else NC
    nc = bacc.Bacc("TRN2", target_bir_lowering=False, debug=False,
                   num_devices=ndev)

    xT_d = nc.declare_dram_parameter("xT", [128, PB, KB * 128], BF16, False)
    W1_d = nc.declare_dram_parameter("W1u", [KB, 128, V1], BF16, False)
    W2_d = nc.declare_dram_parameter("W2u", [HID, V2], BF16, False)
    b1r_d = nc.declare_dram_parameter("b1r", [128, HID], F32, False)
    b2r_d = nc.declare_dram_parameter("b2r", [128, HID2], F32, False)
    Wl_d = nc.declare_dram_parameter("WlBl", [HID2 + 1, OUT], F32, False)
    Pp_d = nc.declare_dram_parameter("Pp", [128, PB * G], BF16, False)
    widx_d = nc.declare_dram_parameter("widx", [128, S * GCHUNK], I16, False)
    sent1_d = nc.declare_dram_parameter("sent1", [1, R], BF16, False)
    sent2_d = nc.declare_dram_parameter("sent2", [1, R], BF16, False)
    out_d = nc.declare_dram_parameter("out", [G, OUT], F32, True)

    shared = dict(addr_space="Shared") if (not sim_mode and NC > 4) else {}
    T1shard = nc.dram_tensor("T1shard", [PCP, R], BF16)
    T1full = nc.dram_tensor("T1full", [TR, R], BF16, **shared)
    T2shard = nc.dram_tensor("T2shard", [PCP, R], BF16)
    T2full = nc.dram_tensor("T2full", [TR, R], BF16, **shared)
    poolin = nc.dram_tensor("poolin", [G, HID2], F32)
    poolout = nc.dram_tensor("poolout", [G, HID2], F32, **shared)

    groups = [list(range(NC))]

    nidx_regs = {}

    def nidx_reg(v):
        if v not in nidx_regs:
            nidx_regs[v] = nc.gpsimd.to_reg(v)
        return nidx_regs[v]

    def gather_block(Gt, Tfull, widx_sb, b, dst0=0):
        L = Ls[b]
        c0 = 0
        while c0 < L:
            CB = min(GCHUNK, L - c0)
            Gvc = Gt[:, (dst0 + c0) * R:(dst0 + c0 + CB) * R].rearrange(
                "p (c e) -> p c e", e=R)
            w0 = (offs[b] + c0) * GCHUNK
            nc.gpsimd.dma_gather(
                out_ap=Gvc, in_ap=Tfull[:],
                idxs_ap=widx_sb[:, w0:w0 + CB * GCHUNK],
                num_idxs=CB * 128, num_idxs_reg=nidx_reg(CB * 128),
                elem_size=R)
            c0 += CB

    with TileContext(nc) as tc:
        with (
            tc.tile_pool(name="const", bufs=1) as cp,
            tc.tile_pool(name="work", bufs=3) as wp,
            tc.tile_pool(name="gath", bufs=2) as gp,
            tc.tile_pool(name="rsp", bufs=1) as rsp,
            tc.tile_pool(name="psA", bufs=2, space="PSUM") as psA,
            tc.tile_pool(name="psB", bufs=2, space="PSUM") as psB,
            tc.tile_pool(name="psP", bufs=1, space="PSUM") as psP,
        ):
            # ---------------- constants to SBUF ----------------
            W1_sb = cp.tile([128, KB * V1], BF16, tag="w1")
            W1v = W1_sb[:].rearrange("p (k h) -> p k h", h=V1)
            nc.sync.dma_start(out=W1v, in_=W1_d[:].rearrange("k p h -> p k h"))
            W2_sb = cp.tile([HID, V2], BF16, tag="w2")
            nc.sync.dma_start(out=W2_sb[:], in_=W2_d[:])
            b1r_sb = cp.tile([128, HID], F32, tag="b1r")
            nc.sync.dma_start(out=b1r_sb[:], in_=b1r_d[:])
            b2r_sb = cp.tile([128, HID2], F32, tag="b2r")
            nc.sync.dma_start(out=b2r_sb[:], in_=b2r_d[:])
            Wl_sb = cp.tile([HID2 + 1, OUT], F32, tag="wl")
            nc.sync.dma_start(out=Wl_sb[:], in_=Wl_d[:])
            P_sb = cp.tile([128, PB * G], BF16, tag="pp")
            nc.sync.dma_start(out=P_sb[:], in_=Pp_d[:])
            widx_sb = cp.tile([128, S * GCHUNK], I16, tag="widx")
            nc.sync.dma_start(out=widx_sb[:], in_=widx_d[:])
            xall = cp.tile([128, PB * KB * 128], BF16, tag="xall")
            nc.sync.dma_start(
                out=xall[:].rearrange("p (c k) -> p c k", k=KB * 128),
                in_=xT_d[:])
            identF = cp.tile([G, G], F32, tag="idf")
            make_identity(nc, identF[:])

            acc1 = cp.tile([128, PB * HID], F32, tag="acc1")
            ob1 = cp.tile([128, PB * HID], F32, tag="ob1")
            r1ball = cp.tile([128, PB * HID], BF16, tag="r1ball")
            dens1 = cp.tile([128, PB], F32, tag="dens1")
            rec1 = cp.tile([128, PB], F32, tag="rec1")
            acc2 = cp.tile([128, PB * HID2], F32, tag="acc2")
            ob2all = cp.tile([128, PB * HID2], F32, tag="ob2all")
            r2ball = cp.tile([128, PB * HID2], BF16, tag="r2ball")
            dens2 = cp.tile([128, PB], F32, tag="dens2")
            rec2 = cp.tile([128, PB], F32, tag="rec2")
            T1sb = cp.tile([128, PB * R], BF16, tag="t1")
            T1sb8 = T1sb[:].bitcast(F8E4)        # [128, PB*256] fp8 view
            T2sb = cp.tile([128, PB * R], BF16, tag="t2")
            nc.vector.memset(T2sb[:], 0.0)
            sd1 = cp.tile([128, PB], F32, tag="sd1")

            pra, prb, prc = phase_reps or (reps, reps, reps)
            for _rep in range(max(pra, prb, prc)):
                # ------- phase A: [h|ss|sd] per node, build T1 rows -------
                for c in range(PB if _rep < pra else 0):
                    ph = psA.tile([128, V1], F32, tag="acc")
                    for kb in range(KB):
                        nc.tensor.matmul(
                            ph[:],
                            lhsT=xall[:, (c * KB + kb) * 128:
                                      (c * KB + kb + 1) * 128],
                            rhs=W1_sb[:, kb * V1:(kb + 1) * V1],
                            start=(kb == 0), stop=(kb == KB - 1),
                        )
                    nc.vector.tensor_copy(
                        T1sb[:, c * R:c * R + 127], ph[:, 0:127])
                    nc.vector.tensor_copy(
                        T1sb8[:, c * 256 + 254:c * 256 + 256],
                        ph[:, 127:129])
                    nc.vector.tensor_copy(sd1[:, c:c + 1], ph[:, 129:130])
                nc.sync.dma_start(
                    out=T1shard[:].rearrange("(c p) w -> p c w", p=128),
                    in_=T1sb[:].rearrange("p (c w) -> p c w", w=R))

                if _rep == 0 or rep_colls:
                    nc.sync.dma_start(out=T1full[TR - 1:TR, :],
                                      in_=sent1_d[:])
                    if sim_mode:
                        nc.sync.dma_start(out=T1full[0:PCP, :],
                                          in_=T1shard[:])
                    else:
                        nc.gpsimd.collective_compute(
                            "AllGather", ALU.bypass, replica_groups=groups,
                            ins=[T1shard[:]], outs=[T1full[0:TR - 1, :]])

                # ------------- phase B: GAT layer 1 + T2 build -------------
                nb = PB if _rep < prb else 0
                for b in range(0, nb, 2):
                    L = Ls[b]
                    Gt = gp.tile([128, 2 * L * R], BF16, tag="g1")
                    Gv = Gt[:].rearrange("p (l w) -> p l w", w=R)
                    Gt8 = Gt[:].bitcast(F8E4).rearrange(
                        "p (l w) -> p l w", w=256)
                    gather_block(Gt, T1full, widx_sb, b)
                    gather_block(Gt, T1full, widx_sb, b + 1, dst0=L)
                    # e = lrelu(ss_src + sd_dst); w = exp(e); den = sum_j w
                    t_t = wp.tile([128, 2 * L], F32, tag="tpre")
                    nc.vector.tensor_tensor(
                        out=t_t[:].rearrange("p (b l) -> p b l", l=L),
                        in0=Gv[:, :, 126].rearrange("p (b l) -> p b l", l=L),
                        in1=sd1[:, b:b + 2, None].to_broadcast([128, 2, L]),
                        op=ALU.add)
                    wl_t = wp.tile([128, 2 * L], F32, tag="wl1")
                    nc.vector.scalar_tensor_tensor(
                        out=wl_t[:], in0=t_t[:], scalar=NEG_SLOPE,
                        in1=t_t[:], op0=ALU.mult, op1=ALU.max)
                    wex = wp.tile([128, 2 * L], F32, tag="we1")
                    nc.scalar.activation(wex[:], wl_t[:], AF.Exp)
                    nc.vector.tensor_reduce(
                        out=dens1[:, b:b + 2],
                        in_=wex[:].rearrange("p (b l) -> p b l", l=L),
                        axis=mybir.AxisListType.X, op=ALU.add)
                    # weighted rows + reduction over slots
                    rs = rsp.tile([128, 2 * L * R], BF16, tag="rs")
                    rs3 = rs[:].rearrange("p (l w) -> p l w", w=R)
                    nc.vector.tensor_tensor(
                        out=rs3[:, :, 0:126], in0=Gv[:, :, 0:126],
                        in1=wex[:, :, None].to_broadcast([128, 2 * L, 126]),
                        op=ALU.mult)
                    nc.vector.tensor_tensor(
                        out=rs3[:, :, 126:128], in0=Gt8[:, :, 254:256],
                        in1=wex[:, :, None].to_broadcast([128, 2 * L, 2]),
                        op=ALU.mult)
                    nc.vector.tensor_reduce(
                        out=acc1[:, b * HID:(b + 2) * HID]
                        .rearrange("p (b f) -> p b f", f=HID),
                        in_=rs[:].rearrange("p (b l f) -> p b f l", f=R, l=L),
                        axis=mybir.AxisListType.X, op=ALU.add)
                if nb:
                    nc.vector.reciprocal(rec1[:], dens1[:])
                    for b in range(nb):
                        nc.vector.scalar_tensor_tensor(
                            out=ob1[:, b * HID:(b + 1) * HID],
                            in0=acc1[:, b * HID:(b + 1) * HID],
                            scalar=rec1[:, b:b + 1],
                            in1=b1r_sb[:], op0=ALU.mult, op1=ALU.add)
                    nc.vector.tensor_scalar(
                        out=r1ball[:], in0=ob1[:], scalar1=0.0, scalar2=None,
                        op0=ALU.max)
                    for b in range(nb):
                        r1T = wp.tile([128, HID], BF16, tag="r1T")
                        nc.sync.dma_start_transpose(
                            r1T[:], r1ball[:, b * HID:(b + 1) * HID])
                        ph2 = psB.tile([128, V2], F32, tag="tr")
                        nc.tensor.matmul(ph2[:], lhsT=r1T[:], rhs=W2_sb[:],
                                         start=True, stop=True)
                        nc.vector.tensor_copy(
                            T2sb[:, b * R:b * R + V2], ph2[:])
                nc.sync.dma_start(
                    out=T2shard[:].rearrange("(c p) w -> p c w", p=128),
                    in_=T2sb[:].rearrange("p (c w) -> p c w", w=R))

                if _rep == 0 or rep_colls:
                    nc.sync.dma_start(out=T2full[TR - 1:TR, :],
                                      in_=sent2_d[:])
                    if sim_mode:
                        nc.sync.dma_start(out=T2full[0:PCP, :],
                                          in_=T2shard[:])
                    else:
                        nc.gpsimd.collective_compute(
                            "AllGather", ALU.bypass, replica_groups=groups,
                            ins=[T2shard[:]], outs=[T2full[0:TR - 1, :]])

                # ------------- phase C: GAT layer 2 + pooling -------------
                if _rep == 0:
                    pool_ps = psP.tile([G, HID2], F32, tag="pool")
                ncb = PB if _rep < prc else 0
                for b in range(0, ncb, 2):
                    L = Ls[b]
                    Gt = gp.tile([128, 2 * L * R], BF16, tag="g1")
                    Gv = Gt[:].rearrange("p (l w) -> p l w", w=R)
                    gather_block(Gt, T2full, widx_sb, b)
                    gather_block(Gt, T2full, widx_sb, b + 1, dst0=L)
                    t_t = wp.tile([128, 2 * L], F32, tag="tpre")
                    nc.vector.tensor_tensor(
                        out=t_t[:].rearrange("p (b l) -> p b l", l=L),
                        in0=Gv[:, :, HID2].rearrange("p (b l) -> p b l", l=L),
                        in1=T2sb[:].rearrange("p (c w) -> p c w", w=R)
                        [:, b:b + 2, HID2 + 1:HID2 + 2]
                        .to_broadcast([128, 2, L]),
                        op=ALU.add)
                    wl_t = wp.tile([128, 2 * L], F32, tag="wl1")
                    nc.vector.scalar_tensor_tensor(
                        out=wl_t[:], in0=t_t[:], scalar=NEG_SLOPE,
                        in1=t_t[:], op0=ALU.mult, op1=ALU.max)
                    wex = wp.tile([128, 2 * L], F32, tag="we1")
                    nc.scalar.activation(wex[:], wl_t[:], AF.Exp)
                    nc.vector.tensor_reduce(
                        out=dens2[:, b:b + 2],
                        in_=wex[:].rearrange("p (b l) -> p b l", l=L),
                        axis=mybir.AxisListType.X, op=ALU.add)
                    rs = rsp.tile([128, 2 * L * HID2], BF16, tag="rs")
                    rs3 = rs[:].rearrange("p (l w) -> p l w", w=HID2)
                    nc.vector.tensor_tensor(
                        out=rs3[:], in0=Gv[:, :, 0:HID2],
                        in1=wex[:, :, None].to_broadcast([128, 2 * L, HID2]),
                        op=ALU.mult)
                    nc.vector.tensor_reduce(
                        out=acc2[:, b * HID2:(b + 2) * HID2]
                        .rearrange("p (b f) -> p b f", f=HID2),
                        in_=rs[:].rearrange("p (b l f) -> p b f l",
                                            f=HID2, l=L),
                        axis=mybir.AxisListType.X, op=ALU.add)
                if ncb:
                    nc.vector.reciprocal(rec2[:], dens2[:])
                    for b in range(ncb):
                        nc.vector.scalar_tensor_tensor(
                            out=ob2all[:, b * HID2:(b + 1) * HID2],
                            in0=acc2[:, b * HID2:(b + 1) * HID2],
                            scalar=rec2[:, b:b + 1],
                            in1=b2r_sb[:], op0=ALU.mult, op1=ALU.add)
                    nc.vector.tensor_scalar(
                        out=r2ball[:], in0=ob2all[:], scalar1=0.0,
                        scalar2=None, op0=ALU.max)
                    for b in range(ncb):
                        nc.tensor.matmul(
                            pool_ps[:], lhsT=P_sb[:, b * G:(b + 1) * G],
                            rhs=r2ball[:, b * HID2:(b + 1) * HID2],
                            start=(b == 0), stop=(b == PB - 1))

            pooled = wp.tile([G, HID2], F32, tag="pool")
            nc.vector.tensor_copy(pooled[:], pool_ps[:])
            nc.sync.dma_start(out=poolin[:], in_=pooled[:])
            if sim_mode:
                nc.sync.dma_start(out=poolout[:], in_=poolin[:])
            else:
                nc.gpsimd.collective_compute(
                    "AllReduce", ALU.add, replica_groups=groups,
                    ins=[poolin[:]], outs=[poolout[:]])
            pooled_r = wp.tile([G, HID2], F32, tag="poolr")
            nc.sync.dma_start(out=pooled_r[:], in_=poolout[:])
            pTf = psB.tile([HID2, G], F32, tag="tr")
            nc.tensor.transpose(pTf[:], pooled_r[:], identity=identF[:])
            fin = wp.tile([HID2 + 1, G], F32, tag="fin")
            nc.vector.tensor_copy(fin[:HID2, :], pTf[:])
            nc.vector.memset(fin[HID2:HID2 + 1, :], 1.0)
            out_ps = psB.tile([G, OUT], F32, tag="tr")
            nc.tensor.matmul(out_ps[:], lhsT=fin[:], rhs=Wl_sb[:],
                             start=True, stop=True)
            out_sb = wp.tile([G, OUT], F32, tag="outsb")
            nc.vector.tensor_copy(out_sb[:], out_ps[:])
            nc.sync.dma_start(out=out_d[:], in_=out_sb[:])

    nc.compile()
    return nc


# ----------------------------------------------------------------------------
# Entry point
# ----------------------------------------------------------------------------

LAST_RESULTS = None


def kernel(**inputs):
    global LAST_RESULTS
    cfg = full_cfg()
    in_maps, meta = preprocess(cfg=cfg, **inputs)
    nc = build_program(cfg, meta)
    res = run_bass_kernel_spmd(nc, in_maps, core_ids=list(range(cfg["NCORES"])))
    LAST_RESULTS = res
    return np.asarray(res.results[0]["out"], np.float32)


# revision 17
# speedup vs baseline: 273.9101x; 1.6359x over previous
"""Trainium2 Bass kernel for a 2-layer GAT + global mean pool + linear head.

Math (matches PyG GATConv, eval mode, single head, add_self_loops=True):
  h   = x @ W
  e_k = lrelu(ss[src_k] + sd[dst_k]),  ss = h@a_src, sd = h@a_dst
  alpha = softmax over incoming edges of each dst (self-loop included)
  out[d] = sum_k alpha_k h[src_k] + b
Two GAT layers (512->128, 128->64) with ReLU, then per-graph mean pool
over `batch` and a final [64,2] linear.

Strategy (8 NeuronCores, full inputs in / full output out):
  * Destination nodes sharded across cores (2500/core), sources arbitrary.
  * Weight matrices extended with W@a_src / W@a_dst columns so one dense
    matmul yields [h | ss | sd] per node.
  * Each core publishes a 256-byte gather-table row per node and
    AllGathers the table:
      layer-1 row = [h[0:126] bf16 | ss bf16 | h[126:128] fp8e4m3]
      layer-2 row = [h2 bf16 | ss2 bf16 | sd2 bf16 | 0-pad]
    256B rows hit dma_gather's fastest descriptor size.
  * Edges grouped per destination into fixed "slots" (sentinel-padded)
    destination-per-partition.  SWDGE dma_gather fetches slot source
    rows in 1024-descriptor chunks (the hardware per-instruction cap).
  * Per 128-destination block the whole attention layer is a handful of
    wide fused ops: ACT bias-add, one fused lrelu (scalar_tensor_tensor
    max(0.2t, t)), ACT Exp with accum_out as the softmax denominator,
    one broadcast-multiply, one strided tensor_reduce.  This matters
    because the execution environment has a large per-instruction
    dispatch overhead, so wall time ~ instruction count.
  * Per-graph pooling one-hots (1/count folded in) are host-built bf16;
    partial pooled features are AllReduced, final linear on every core.

All graph-structure preprocessing (degree sort, slot layout, index
remapping, 16-partition index wrapping) is host-side numpy on the
kernel inputs; the device only sees dense arrays.
"""

import math
import numpy as np

import concourse.bass as bass
import concourse.bacc as bacc
import concourse.mybir as mybir
from concourse.tile import TileContext
from concourse.masks import make_identity
from concourse.bass_utils import run_bass_kernel_spmd

F32 = mybir.dt.float32
BF16 = mybir.dt.bfloat16
F8E4 = mybir.dt.float8e4
I16 = mybir.dt.int16
AF = mybir.ActivationFunctionType
ALU = mybir.AluOpType

NEG_SLOPE = 0.2
SENT_SS = -60.0  # sentinel row score: exp(lrelu(-60+sd)) ~ e^-11 -> harmless
GCHUNK = 8       # slot columns per dma_gather (8*128 = 1024 descriptors)


def full_cfg():
    return dict(N=20000, IND=512, HID=128, HID2=64, OUT=2, G=16, NCORES=8)


# ----------------------------------------------------------------------------
# Host-side preprocessing
# ----------------------------------------------------------------------------

def preprocess(x, edge_index, batch, W1, a1_src, a1_dst, b1,
               W2, a2_src, a2_dst, b2, Wl, bl, cfg):
    N, IND, HID, HID2, OUT, G, NC = (cfg[k] for k in
                                     ("N", "IND", "HID", "HID2", "OUT", "G",
                                      "NCORES"))
    PC = math.ceil(N / NC)            # real dests per core
    PB = math.ceil(PC / 128)          # dest blocks per core
    PCP = PB * 128                    # padded dests per core
    TR = NC * PCP + 1                 # table rows (+1 sentinel)
    SENT = TR - 1
    KB = IND // 128
    R = 128                           # table row: 128 bf16 elems = 256B

    import ml_dtypes
    BF = ml_dtypes.bfloat16

    x = np.asarray(x, np.float32)
    batch = np.asarray(batch, np.int64)
    src = np.asarray(edge_index[0], np.int64)
    dst = np.asarray(edge_index[1], np.int64)
    # self loops
    loop = np.arange(N, dtype=np.int64)
    src = np.concatenate([src, loop])
    dst = np.concatenate([dst, loop])

    counts = np.bincount(batch, minlength=G).astype(np.float64)

    # per-core degree-sorted permutations and global row ids
    row_of = np.empty(N, np.int64)       # global node -> table row
    orders = []
    degs_sorted = np.zeros((NC, PCP), np.int64)
    for k in range(NC):
        lo, hi = k * PC, min((k + 1) * PC, N)
        nk = hi - lo
        mask = (dst >= lo) & (dst < hi)
        deg = np.bincount(dst[mask] - lo, minlength=nk)
        order = np.argsort(-deg, kind="stable")        # local rank -> local id
        inv = np.empty(nk, np.int64)
        inv[order] = np.arange(nk)
        row_of[lo:hi] = k * PCP + inv
        orders.append(order)
        degs_sorted[k, :nk] = deg[order]

    # global per-block slot counts (identical program on every core),
    # padded to be equal within each pair of consecutive blocks so both
    # blocks of a pair can share one set of fused score/aggregation ops
    Ls = []
    for b in range(PB):
        Lb = int(degs_sorted[:, b * 128:(b + 1) * 128].max())
        Ls.append(max(Lb, 1))
    for i in range(0, PB - 1, 2):
        Lp = max(Ls[i], Ls[i + 1])
        Ls[i] = Ls[i + 1] = Lp
    S = int(np.sum(Ls))
    offs = np.concatenate([[0], np.cumsum(Ls)]).astype(np.int64)

    # extended weights; layer-1 psum order [h0:126 | ss | h126 | h127 | sd]
    # so the table row [h0:126 | ss] is one contiguous psum copy.
    W1f = np.asarray(W1, np.float32)
    W1e = np.concatenate([
        W1f[:, 0:126],
        (W1f @ np.asarray(a1_src, np.float32))[:, None],
        W1f[:, 126:128],
        (W1f @ np.asarray(a1_dst, np.float32))[:, None],
    ], axis=1)                                   # [512, 130]
    W1u = np.ascontiguousarray(W1e.reshape(KB, 128, HID + 2)).astype(BF)
    W2f = np.asarray(W2, np.float32)
    W2e = np.concatenate([
        W2f,
        (W2f @ np.asarray(a2_src, np.float32))[:, None],
        (W2f @ np.asarray(a2_dst, np.float32))[:, None],
    ], axis=1).astype(BF)                        # [128, 66] = [h2|ss2|sd2]
    b1r = np.tile(np.asarray(b1, np.float32)[None, :], (128, 1))
    b2r = np.tile(np.asarray(b2, np.float32)[None, :], (128, 1))
    WlBl = np.concatenate([np.asarray(Wl, np.float32),
                           np.asarray(bl, np.float32)[None, :]], axis=0)
    sent = np.zeros((1, R), BF)
    sent[0, 126] = SENT_SS        # layer-1 sentinel: ss at elem 126
    sent2 = np.zeros((1, R), BF)
    sent2[0, HID2] = SENT_SS      # layer-2 sentinel: ss2 at elem 64

    in_maps = []
    for k in range(NC):
        lo, hi = k * PC, min((k + 1) * PC, N)
        nk = hi - lo
        order = orders[k]

        # xT: [128, PB, KB*128] partition = feature-within-chunk, so
        # xT[p, c, kb*128+m] = xs[c*128+m, kb*128+p]  (1KB contiguous/desc)
        xs = np.zeros((PCP, IND), np.float32)
        xs[:nk] = x[lo:hi][order]
        xT = np.ascontiguousarray(
            xs.reshape(PB, 128, KB, 128).transpose(3, 0, 2, 1)
            .reshape(128, PB, KB * 128)).astype(ml_dtypes.float8_e4m3)

        # slot indices [128, S] -> table rows, sentinel padded
        sidx = np.full((128, S), SENT, np.int64)
        mask = (dst >= lo) & (dst < hi)
        es, ed = src[mask], dst[mask] - lo
        o = np.argsort(ed, kind="stable")
        es, ed = es[o], ed[o]
        deg = np.bincount(ed, minlength=nk)
        start = np.concatenate([[0], np.cumsum(deg)[:-1]])
        j = np.arange(len(ed)) - start[ed]            # slot within dest
        inv = np.empty(nk, np.int64)
        inv[order] = np.arange(nk)
        r = inv[ed]                                   # dest rank
        bb, pp = r // 128, r % 128
        col = offs[bb] + j
        sidx[pp, col] = row_of[es]

        # dma_gather wrapped indices: per chunk of <=GCHUNK slot columns,
        # flat[i] lands at out partition i%128, column i//128;
        # idxs[p, c] = flat[c*16 + p%16], replicated over the 8 q7 cores.
        wcols = []
        for b in range(PB):
            L = Ls[b]
            for c0 in range(0, L, GCHUNK):
                CB = min(GCHUNK, L - c0)
                flat = sidx[:, offs[b] + c0: offs[b] + c0 + CB].T.reshape(-1)
                wcols.append(flat.reshape(-1, 16).T)
        widx = np.concatenate(wcols, axis=1).astype(np.int16)  # [16, S*8]

        # pooling one-hot with 1/count folded, zero rows for pad dests
        P = np.zeros((128, PB * G), np.float32)
        bg = batch[lo:hi][order]                      # graph id per rank
        rr = np.arange(nk)
        P[rr % 128, (rr // 128) * G + bg] = 1.0 / np.maximum(counts[bg], 1.0)

        in_maps.append(dict(
            xT=xT, W1u=W1u, W2u=W2e, b1r=b1r, b2r=b2r,
            WlBl=WlBl.astype(np.float32),
            Pp=P.astype(BF), widx=widx, sent1=sent, sent2=sent2,
        ))

    meta = dict(PC=PC, PB=PB, PCP=PCP, TR=TR, R=R, KB=KB, S=S,
                Ls=Ls, offs=offs)
    return in_maps, meta


# ----------------------------------------------------------------------------
# Device program
# ----------------------------------------------------------------------------

def build_program(cfg, meta, sim_mode=False, reps=1, rep_colls=True, phase_reps=None):
    N, IND, HID, HID2, OUT, G, NC = (cfg[k] for k in
                                     ("N", "IND", "HID", "HID2", "OUT",
                                      "G", "NCORES"))
    PB, PCP, TR, R, KB, S = (meta[k] for k in
                             ("PB", "PCP", "TR", "R", "KB", "S"))
    Ls, offs = meta["Ls"], meta["offs"]
    V1 = HID + 2        # phase-A psum width  [h0:126 | ss | h126:128 | sd]
    V2 = HID2 + 2

    ndev = 1 if sim_mode else NC
    nc = bacc.Bacc("TRN2", target_bir_lowering=False, debug=False,
                   num_devices=ndev)

    xT_d = nc.declare_dram_parameter("xT", [128, PB, KB * 128], F8E4, False)
    W1_d = nc.declare_dram_parameter("W1u", [KB, 128, V1], BF16, False)
    W2_d = nc.declare_dram_parameter("W2u", [HID, V2], BF16, False)
    b1r_d = nc.declare_dram_parameter("b1r", [128, HID], F32, False)
    b2r_d = nc.declare_dram_parameter("b2r", [128, HID2], F32, False)
    Wl_d = nc.declare_dram_parameter("WlBl", [HID2 + 1, OUT], F32, False)
    Pp_d = nc.declare_dram_parameter("Pp", [128, PB * G], BF16, False)
    widx_d = nc.declare_dram_parameter("widx", [16, S * GCHUNK], I16, False)
    sent1_d = nc.declare_dram_parameter("sent1", [1, R], BF16, False)
    sent2_d = nc.declare_dram_parameter("sent2", [1, R], BF16, False)
    out_d = nc.declare_dram_parameter("out", [G, OUT], F32, True)

    shared = dict(addr_space="Shared") if (not sim_mode and NC > 4) else {}
    T1shard = nc.dram_tensor("T1shard", [PCP, R], BF16)
    T1full = nc.dram_tensor("T1full", [TR, R], BF16, **shared)
    T2shard = nc.dram_tensor("T2shard", [PCP, R], BF16)
    T2full = nc.dram_tensor("T2full", [TR, R], BF16, **shared)
    poolin = nc.dram_tensor("poolin", [G, HID2], F32)
    poolout = nc.dram_tensor("poolout", [G, HID2], F32, **shared)

    groups = [list(range(NC))]

    nidx_regs = {}

    def nidx_reg(v):
        if v not in nidx_regs:
            nidx_regs[v] = nc.gpsimd.to_reg(v)
        return nidx_regs[v]

    def gather_block(Gt, Tfull, widx_sb, b, dst0=0):
        L = Ls[b]
        c0 = 0
        while c0 < L:
            CB = min(GCHUNK, L - c0)
            Gvc = Gt[:, (dst0 + c0) * R:(dst0 + c0 + CB) * R].rearrange(
                "p (c e) -> p c e", e=R)
            w0 = (offs[b] + c0) * GCHUNK
            nc.gpsimd.dma_gather(
                out_ap=Gvc, in_ap=Tfull[:],
                idxs_ap=widx_sb[:, w0:w0 + CB * GCHUNK],
                num_idxs=CB * 128, num_idxs_reg=nidx_reg(CB * 128),
                elem_size=R)
            c0 += CB

    with TileContext(nc) as tc:
        with (
            tc.tile_pool(name="const", bufs=1) as cp,
            tc.tile_pool(name="work", bufs=3) as wp,
            tc.tile_pool(name="gath", bufs=2) as gp,
            tc.tile_pool(name="rsp", bufs=1) as rsp,
            tc.tile_pool(name="psA", bufs=2, space="PSUM") as psA,
            tc.tile_pool(name="psB", bufs=2, space="PSUM") as psB,
            tc.tile_pool(name="psP", bufs=1, space="PSUM") as psP,
        ):
            # ---------------- constants to SBUF ----------------
            W1_sb = cp.tile([128, KB * V1], BF16, tag="w1")
            W1v = W1_sb[:].rearrange("p (k h) -> p k h", h=V1)
            nc.sync.dma_start(out=W1v, in_=W1_d[:].rearrange("k p h -> p k h"))
            W2_sb = cp.tile([HID, V2], BF16, tag="w2")
            nc.sync.dma_start(out=W2_sb[:], in_=W2_d[:])
            b1r_sb = cp.tile([128, HID], F32, tag="b1r")
            nc.sync.dma_start(out=b1r_sb[:], in_=b1r_d[:])
            b2r_sb = cp.tile([128, HID2], F32, tag="b2r")
            nc.sync.dma_start(out=b2r_sb[:], in_=b2r_d[:])
            Wl_sb = cp.tile([HID2 + 1, OUT], F32, tag="wl")
            nc.sync.dma_start(out=Wl_sb[:], in_=Wl_d[:])
            P_sb = cp.tile([128, PB * G], BF16, tag="pp")
            nc.sync.dma_start(out=P_sb[:], in_=Pp_d[:])
            widx_sb = cp.tile([128, S * GCHUNK], I16, tag="widx")
            nc.sync.dma_start(out=widx_sb[0:16, :], in_=widx_d[:])
            nc.sync.dma_start(out=widx_sb[16:32, :], in_=widx_sb[0:16, :])
            nc.sync.dma_start(out=widx_sb[32:64, :], in_=widx_sb[0:32, :])
            nc.sync.dma_start(out=widx_sb[64:128, :], in_=widx_sb[0:64, :])
            xall = cp.tile([128, PB * KB * 128], F8E4, tag="xall")
            nc.sync.dma_start(
                out=xall[:].rearrange("p (c k) -> p c k", k=KB * 128),
                in_=xT_d[:])
            identF = cp.tile([G, G], F32, tag="idf")
            make_identity(nc, identF[:])

            acc1 = cp.tile([128, PB * HID], F32, tag="acc1")
            ob1 = cp.tile([128, PB * HID], F32, tag="ob1")
            r1ball = cp.tile([128, PB * HID], BF16, tag="r1ball")
            dens1 = cp.tile([128, PB], F32, tag="dens1")
            rec1 = cp.tile([128, PB], F32, tag="rec1")
            acc2 = cp.tile([128, PB * HID2], F32, tag="acc2")
            ob2all = cp.tile([128, PB * HID2], F32, tag="ob2all")
            r2ball = cp.tile([128, PB * HID2], BF16, tag="r2ball")
            dens2 = cp.tile([128, PB], F32, tag="dens2")
            rec2 = cp.tile([128, PB], F32, tag="rec2")
            T1sb = cp.tile([128, PB * R], BF16, tag="t1")
            T1sb8 = T1sb[:].bitcast(F8E4)        # [128, PB*256] fp8 view
            T2sb = cp.tile([128, PB * R], BF16, tag="t2")
            nc.vector.memset(T2sb[:], 0.0)
            sd1 = cp.tile([128, PB], F32, tag="sd1")

            pra, prb, prc = phase_reps or (reps, reps, reps)
            for _rep in range(max(pra, prb, prc)):
                # ------- phase A: [h|ss|sd] per node, build T1 rows -------
                for c in range(PB if _rep < pra else 0):
                    ph = psA.tile([128, V1], F32, tag="acc")
                    for kb in range(KB):
                        nc.tensor.matmul(
                            ph[:],
                            lhsT=xall[:, (c * KB + kb) * 128:
                                      (c * KB + kb + 1) * 128],
                            rhs=W1_sb[:, kb * V1:(kb + 1) * V1],
                            start=(kb == 0), stop=(kb == KB - 1),
                        )
                    nc.vector.tensor_copy(
                        T1sb[:, c * R:c * R + 127], ph[:, 0:127])
                    nc.vector.tensor_copy(
                        T1sb8[:, c * 256 + 254:c * 256 + 256],
                        ph[:, 127:129])
                    nc.vector.tensor_copy(sd1[:, c:c + 1], ph[:, 129:130])
                nc.sync.dma_start(
                    out=T1shard[:].rearrange("(c p) w -> p c w", p=128),
                    in_=T1sb[:].rearrange("p (c w) -> p c w", w=R))

                if _rep == 0 or rep_colls:
                    nc.sync.dma_start(out=T1full[TR - 1:TR, :],
                                      in_=sent1_d[:])
                    if sim_mode:
                        nc.sync.dma_start(out=T1full[0:PCP, :],
                                          in_=T1shard[:])
                    else:
                        nc.gpsimd.collective_compute(
                            "AllGather", ALU.bypass, replica_groups=groups,
                            ins=[T1shard[:]], outs=[T1full[0:TR - 1, :]])

                # ------------- phase B: GAT layer 1 + T2 build -------------
                nb = PB if _rep < prb else 0
                for b in range(0, nb, 2):
                    L = Ls[b]
                    Gt = gp.tile([128, 2 * L * R], BF16, tag="g1")
                    Gv = Gt[:].rearrange("p (l w) -> p l w", w=R)
                    Gt8 = Gt[:].bitcast(F8E4).rearrange(
                        "p (l w) -> p l w", w=256)
                    gather_block(Gt, T1full, widx_sb, b)
                    gather_block(Gt, T1full, widx_sb, b + 1, dst0=L)
                    # e = lrelu(ss_src + sd_dst); w = exp(e); den = sum_j w
                    t_t = wp.tile([128, 2 * L], F32, tag="tpre")
                    nc.vector.tensor_tensor(
                        out=t_t[:].rearrange("p (b l) -> p b l", l=L),
                        in0=Gv[:, :, 126].rearrange("p (b l) -> p b l", l=L),
                        in1=sd1[:, b:b + 2, None].to_broadcast([128, 2, L]),
                        op=ALU.add)
                    wl_t = wp.tile([128, 2 * L], F32, tag="wl1")
                    nc.vector.scalar_tensor_tensor(
                        out=wl_t[:], in0=t_t[:], scalar=NEG_SLOPE,
                        in1=t_t[:], op0=ALU.mult, op1=ALU.max)
                    wex = wp.tile([128, 2 * L], F32, tag="we1")
                    nc.scalar.activation(wex[:], wl_t[:], AF.Exp)
                    nc.vector.tensor_reduce(
                        out=dens1[:, b:b + 2],
                        in_=wex[:].rearrange("p (b l) -> p b l", l=L),
                        axis=mybir.AxisListType.X, op=ALU.add)
                    # weighted rows + reduction over slots
                    rs = rsp.tile([128, 2 * L * R], BF16, tag="rs")
                    rs3 = rs[:].rearrange("p (l w) -> p l w", w=R)
                    nc.vector.tensor_tensor(
                        out=rs3[:, :, 0:126], in0=Gv[:, :, 0:126],
                        in1=wex[:, :, None].to_broadcast([128, 2 * L, 126]),
                        op=ALU.mult)
                    nc.vector.tensor_tensor(
                        out=rs3[:, :, 126:128], in0=Gt8[:, :, 254:256],
                        in1=wex[:, :, None].to_broadcast([128, 2 * L, 2]),
                        op=ALU.mult)
                    nc.vector.tensor_reduce(
                        out=acc1[:, b * HID:(b + 2) * HID]
                        .rearrange("p (b f) -> p b f", f=HID),
                        in_=rs[:].rearrange("p (b l f) -> p b f l", f=R, l=L),
                        axis=mybir.AxisListType.X, op=ALU.add)
                if nb:
                    nc.vector.reciprocal(rec1[:], dens1[:])
                    for b in range(nb):
                        nc.vector.scalar_tensor_tensor(
                            out=ob1[:, b * HID:(b + 1) * HID],
                            in0=acc1[:, b * HID:(b + 1) * HID],
                            scalar=rec1[:, b:b + 1],
                            in1=b1r_sb[:], op0=ALU.mult, op1=ALU.add)
                    nc.vector.tensor_scalar(
                        out=r1ball[:], in0=ob1[:], scalar1=0.0, scalar2=None,
                        op0=ALU.max)
                    for b in range(nb):
                        r1T = wp.tile([128, HID], BF16, tag="r1T")
                        nc.sync.dma_start_transpose(
                            r1T[:], r1ball[:, b * HID:(b + 1) * HID])
                        ph2 = psB.tile([128, V2], F32, tag="tr")
                        nc.tensor.matmul(ph2[:], lhsT=r1T[:], rhs=W2_sb[:],
                                         start=True, stop=True)
                        nc.vector.tensor_copy(
                            T2sb[:, b * R:b * R + V2], ph2[:])
                nc.sync.dma_start(
                    out=T2shard[:].rearrange("(c p) w -> p c w", p=128),
                    in_=T2sb[:].rearrange("p (c w) -> p c w", w=R))

                if _rep == 0 or rep_colls:
                    nc.sync.dma_start(out=T2full[TR - 1:TR, :],
                                      in_=sent2_d[:])
                    if sim_mode:
                        nc.sync.dma_start(out=T2full[0:PCP, :],
                                          in_=T2shard[:])
                    else:
                        nc.gpsimd.collective_compute(
                            "AllGather", ALU.bypass, replica_groups=groups,
                            ins=[T2shard[:]], outs=[T2full[0:TR - 1, :]])

                # ------------- phase C: GAT layer 2 + pooling -------------
                if _rep == 0:
                    pool_ps = psP.tile([G, HID2], F32, tag="pool")
                ncb = PB if _rep < prc else 0
                for b in range(0, ncb, 2):
                    L = Ls[b]
                    Gt = gp.tile([128, 2 * L * R], BF16, tag="g1")
                    Gv = Gt[:].rearrange("p (l w) -> p l w", w=R)
                    gather_block(Gt, T2full, widx_sb, b)
                    gather_block(Gt, T2full, widx_sb, b + 1, dst0=L)
                    t_t = wp.tile([128, 2 * L], F32, tag="tpre")
                    nc.vector.tensor_tensor(
                        out=t_t[:].rearrange("p (b l) -> p b l", l=L),
                        in0=Gv[:, :, HID2].rearrange("p (b l) -> p b l", l=L),
                        in1=T2sb[:].rearrange("p (c w) -> p c w", w=R)
                        [:, b:b + 2, HID2 + 1:HID2 + 2]
                        .to_broadcast([128, 2, L]),
                        op=ALU.add)
                    wl_t = wp.tile([128, 2 * L], F32, tag="wl1")
                    nc.vector.scalar_tensor_tensor(
                        out=wl_t[:], in0=t_t[:], scalar=NEG_SLOPE,
                        in1=t_t[:], op0=ALU.mult, op1=ALU.max)
                    wex = wp.tile([128, 2 * L], F32, tag="we1")
                    nc.scalar.activation(wex[:], wl_t[:], AF.Exp)
                    nc.vector.tensor_reduce(
                        out=dens2[:, b:b + 2],
                        in_=wex[:].rearrange("p (b l) -> p b l", l=L),
                        axis=mybir.AxisListType.X, op=ALU.add)
                    rs = rsp.tile([128, 2 * L * HID2], BF16, tag="rs")
                    rs3 = rs[:].rearrange("p (l w) -> p l w", w=HID2)
                    nc.vector.tensor_tensor(
                        out=rs3[:], in0=Gv[:, :, 0:HID2],
                        in1=wex[:, :, None].to_broadcast([128, 2 * L, HID2]),
                        op=ALU.mult)
                    nc.vector.tensor_reduce(
                        out=acc2[:, b * HID2:(b + 2) * HID2]
                        .rearrange("p (b f) -> p b f", f=HID2),
                        in_=rs[:].rearrange("p (b l f) -> p b f l",
                                            f=HID2, l=L),
                        axis=mybir.AxisListType.X, op=ALU.add)
                if ncb:
                    nc.vector.reciprocal(rec2[:], dens2[:])
                    for b in range(ncb):
                        nc.vector.scalar_tensor_tensor(
                            out=ob2all[:, b * HID2:(b + 1) * HID2],
                            in0=acc2[:, b * HID2:(b + 1) * HID2],
                            scalar=rec2[:, b:b + 1],
                            in1=b2r_sb[:], op0=ALU.mult, op1=ALU.add)
                    nc.vector.tensor_scalar(
                        out=r2ball[:], in0=ob2all[:], scalar1=0.0,
                        scalar2=None, op0=ALU.max)
                    for b in range(ncb):
                        nc.tensor.matmul(
                            pool_ps[:], lhsT=P_sb[:, b * G:(b + 1) * G],
                            rhs=r2ball[:, b * HID2:(b + 1) * HID2],
                            start=(b == 0), stop=(b == PB - 1))

            pooled = wp.tile([G, HID2], F32, tag="pool")
            nc.vector.tensor_copy(pooled[:], pool_ps[:])
            nc.sync.dma_start(out=poolin[:], in_=pooled[:])
            if sim_mode:
                nc.sync.dma_start(out=poolout[:], in_=poolin[:])
            else:
                nc.gpsimd.collective_compute(
                    "AllReduce", ALU.add, replica_groups=groups,
                    ins=[poolin[:]], outs=[poolout[:]])
            pooled_r = wp.tile([G, HID2], F32, tag="poolr")
            nc.sync.dma_start(out=pooled_r[:], in_=poolout[:])
            pTf = psB.tile([HID2, G], F32, tag="tr")
            nc.tensor.transpose(pTf[:], pooled_r[:], identity=identF[:])
            fin = wp.tile([HID2 + 1, G], F32, tag="fin")
            nc.vector.tensor_copy(fin[:HID2, :], pTf[:])
            nc.vector.memset(fin[HID2:HID2 + 1, :], 1.0)
            out_ps = psB.tile([G, OUT], F32, tag="tr")
            nc.tensor.matmul(out_ps[:], lhsT=fin[:], rhs=Wl_sb[:],
                             start=True, stop=True)
            out_sb = wp.tile([G, OUT], F32, tag="outsb")
            nc.vector.tensor_copy(out_sb[:], out_ps[:])
            nc.sync.dma_start(out=out_d[:], in_=out_sb[:])

    nc.compile()
    return nc


# ----------------------------------------------------------------------------
# Entry point
# ----------------------------------------------------------------------------

LAST_RESULTS = None


def kernel(**inputs):
    global LAST_RESULTS
    cfg = full_cfg()
    in_maps, meta = preprocess(cfg=cfg, **inputs)
    nc = build_program(cfg, meta)
    res = run_bass_kernel_spmd(nc, in_maps, core_ids=list(range(cfg["NCORES"])))
    LAST_RESULTS = res
    return np.asarray(res.results[0]["out"], np.float32)


# revision 18
# speedup vs baseline: 284.9483x; 1.0403x over previous
"""Trainium2 Bass kernel for a 2-layer GAT + global mean pool + linear head.

Math (matches PyG GATConv, eval mode, single head, add_self_loops=True):
  h   = x @ W
  e_k = lrelu(ss[src_k] + sd[dst_k]),  ss = h@a_src, sd = h@a_dst
  alpha = softmax over incoming edges of each dst (self-loop included)
  out[d] = sum_k alpha_k h[src_k] + b
Two GAT layers (512->128, 128->64) with ReLU, then per-graph mean pool
over `batch` and a final [64,2] linear.

Strategy (8 NeuronCores, full inputs in / full output out):
  * Destination nodes sharded across cores (2500/core), sources arbitrary.
  * Weight matrices extended with W@a_src / W@a_dst columns so one dense
    matmul yields [h | ss | sd] per node.
  * Each core publishes a 256-byte gather-table row per node and
    AllGathers the table:
      layer-1 row = [h[0:126] bf16 | ss bf16 | h[126:128] fp8e4m3]
      layer-2 row = [h2 bf16 | ss2 bf16 | sd2 bf16 | 0-pad]
    256B rows hit dma_gather's fastest descriptor size.
  * Edges grouped per destination into fixed "slots" (sentinel-padded)
    destination-per-partition.  SWDGE dma_gather fetches slot source
    rows in 1024-descriptor chunks (the hardware per-instruction cap).
  * Per 128-destination block the whole attention layer is a handful of
    wide fused ops: ACT bias-add, one fused lrelu (scalar_tensor_tensor
    max(0.2t, t)), ACT Exp with accum_out as the softmax denominator,
    one broadcast-multiply, one strided tensor_reduce.  This matters
    because the execution environment has a large per-instruction
    dispatch overhead, so wall time ~ instruction count.
  * Per-graph pooling one-hots (1/count folded in) are host-built bf16;
    partial pooled features are AllReduced, final linear on every core.

All graph-structure preprocessing (degree sort, slot layout, index
remapping, 16-partition index wrapping) is host-side numpy on the
kernel inputs; the device only sees dense arrays.
"""

import math
import numpy as np

import concourse.bass as bass
import concourse.bacc as bacc
import concourse.mybir as mybir
from concourse.tile import TileContext
from concourse.masks import make_identity
from concourse.bass_utils import run_bass_kernel_spmd

F32 = mybir.dt.float32
BF16 = mybir.dt.bfloat16
F8E4 = mybir.dt.float8e4
I16 = mybir.dt.int16
AF = mybir.ActivationFunctionType
ALU = mybir.AluOpType

NEG_SLOPE = 0.2
SENT_SS = -60.0  # sentinel row score: exp(lrelu(-60+sd)) ~ e^-11 -> harmless
GCHUNK = 8       # slot columns per dma_gather (8*128 = 1024 descriptors)


def full_cfg():
    return dict(N=20000, IND=512, HID=128, HID2=64, OUT=2, G=16, NCORES=8)


# ----------------------------------------------------------------------------
# Host-side preprocessing
# ----------------------------------------------------------------------------

def preprocess(x, edge_index, batch, W1, a1_src, a1_dst, b1,
               W2, a2_src, a2_dst, b2, Wl, bl, cfg):
    N, IND, HID, HID2, OUT, G, NC = (cfg[k] for k in
                                     ("N", "IND", "HID", "HID2", "OUT", "G",
                                      "NCORES"))
    PC = math.ceil(N / NC)            # real dests per core
    PB = math.ceil(PC / 128)          # dest blocks per core
    PCP = PB * 128                    # padded dests per core
    TR = NC * PCP + 1                 # table rows (+1 sentinel)
    SENT = TR - 1
    KB = IND // 128
    R = 128                           # table row: 128 bf16 elems = 256B

    import ml_dtypes
    BF = ml_dtypes.bfloat16

    x = np.asarray(x, np.float32)
    batch = np.asarray(batch, np.int64)
    src = np.asarray(edge_index[0], np.int64)
    dst = np.asarray(edge_index[1], np.int64)
    # self loops
    loop = np.arange(N, dtype=np.int64)
    src = np.concatenate([src, loop])
    dst = np.concatenate([dst, loop])

    counts = np.bincount(batch, minlength=G).astype(np.float64)

    # per-core degree-sorted permutations and global row ids
    row_of = np.empty(N, np.int64)       # global node -> table row
    orders = []
    degs_sorted = np.zeros((NC, PCP), np.int64)
    for k in range(NC):
        lo, hi = k * PC, min((k + 1) * PC, N)
        nk = hi - lo
        mask = (dst >= lo) & (dst < hi)
        deg = np.bincount(dst[mask] - lo, minlength=nk)
        order = np.argsort(-deg, kind="stable")        # local rank -> local id
        inv = np.empty(nk, np.int64)
        inv[order] = np.arange(nk)
        row_of[lo:hi] = k * PCP + inv
        orders.append(order)
        degs_sorted[k, :nk] = deg[order]

    # global per-block slot counts (identical program on every core),
    # padded to be equal within each pair of consecutive blocks so both
    # blocks of a pair can share one set of fused score/aggregation ops
    Ls = []
    for b in range(PB):
        Lb = int(degs_sorted[:, b * 128:(b + 1) * 128].max())
        Ls.append(max(Lb, 1))
    for i in range(0, PB - 1, 2):
        Lp = max(Ls[i], Ls[i + 1])
        Ls[i] = Ls[i + 1] = Lp
    S = int(np.sum(Ls))
    offs = np.concatenate([[0], np.cumsum(Ls)]).astype(np.int64)

    # extended weights; layer-1 psum order [h0:126 | ss | h126 | h127 | sd]
    # so the table row [h0:126 | ss] is one contiguous psum copy.
    W1f = np.asarray(W1, np.float32)
    W1e = np.concatenate([
        W1f[:, 0:126],
        (W1f @ np.asarray(a1_src, np.float32))[:, None],
        W1f[:, 126:128],
        (W1f @ np.asarray(a1_dst, np.float32))[:, None],
    ], axis=1)                                   # [512, 130]
    W1u = np.ascontiguousarray(W1e.reshape(KB, 128, HID + 2)).astype(BF)
    W2f = np.asarray(W2, np.float32)
    W2e = np.concatenate([
        W2f,
        (W2f @ np.asarray(a2_src, np.float32))[:, None],
        (W2f @ np.asarray(a2_dst, np.float32))[:, None],
    ], axis=1).astype(BF)                        # [128, 66] = [h2|ss2|sd2]
    b1v = np.asarray(b1, np.float32)[None, :]
    b2v = np.asarray(b2, np.float32)[None, :]
    iotaG = np.tile(np.arange(G, dtype=np.float32)[None, :], (128, 1))
    WlBl = np.concatenate([np.asarray(Wl, np.float32),
                           np.asarray(bl, np.float32)[None, :]], axis=0)
    sent = np.zeros((1, R), BF)
    sent[0, 126] = SENT_SS        # layer-1 sentinel: ss at elem 126
    sent2 = np.zeros((1, R), BF)
    sent2[0, HID2] = SENT_SS      # layer-2 sentinel: ss2 at elem 64

    in_maps = []
    for k in range(NC):
        lo, hi = k * PC, min((k + 1) * PC, N)
        nk = hi - lo
        order = orders[k]

        # xT: [128, PB, KB*128] partition = feature-within-chunk, so
        # xT[p, c, kb*128+m] = xs[c*128+m, kb*128+p]  (1KB contiguous/desc)
        xs = np.zeros((PCP, IND), np.float32)
        xs[:nk] = x[lo:hi][order]
        xT = np.ascontiguousarray(
            xs.reshape(PB, 128, KB, 128).transpose(3, 0, 2, 1)
            .reshape(128, PB, KB * 128)).astype(ml_dtypes.float8_e4m3)

        # slot indices [128, S] -> table rows, sentinel padded
        sidx = np.full((128, S), SENT, np.int64)
        mask = (dst >= lo) & (dst < hi)
        es, ed = src[mask], dst[mask] - lo
        o = np.argsort(ed, kind="stable")
        es, ed = es[o], ed[o]
        deg = np.bincount(ed, minlength=nk)
        start = np.concatenate([[0], np.cumsum(deg)[:-1]])
        j = np.arange(len(ed)) - start[ed]            # slot within dest
        inv = np.empty(nk, np.int64)
        inv[order] = np.arange(nk)
        r = inv[ed]                                   # dest rank
        bb, pp = r // 128, r % 128
        col = offs[bb] + j
        sidx[pp, col] = row_of[es]

        # dma_gather wrapped indices: per chunk of <=GCHUNK slot columns,
        # flat[i] lands at out partition i%128, column i//128;
        # idxs[p, c] = flat[c*16 + p%16], replicated over the 8 q7 cores.
        wcols = []
        for b in range(PB):
            L = Ls[b]
            for c0 in range(0, L, GCHUNK):
                CB = min(GCHUNK, L - c0)
                flat = sidx[:, offs[b] + c0: offs[b] + c0 + CB].T.reshape(-1)
                wcols.append(flat.reshape(-1, 16).T)
        widx = np.concatenate(wcols, axis=1).astype(np.int16)  # [16, S*8]

        # per-rank graph id (pad ranks get G -> equality test never fires)
        # and 1/count, for the on-device pooling one-hot construction
        bgq = np.full((128, PB), float(G), np.float32)
        ivc = np.zeros((128, PB), np.float32)
        bg = batch[lo:hi][order]                      # graph id per rank
        rr = np.arange(nk)
        bgq[rr % 128, rr // 128] = bg
        ivc[rr % 128, rr // 128] = 1.0 / np.maximum(counts[bg], 1.0)

        in_maps.append(dict(
            xT=xT, W1u=W1u, W2u=W2e, b1v=b1v, b2v=b2v, iotaG=iotaG,
            WlBl=WlBl.astype(np.float32),
            bgq=bgq, ivc=ivc, widx=widx, sent1=sent, sent2=sent2,
        ))

    meta = dict(PC=PC, PB=PB, PCP=PCP, TR=TR, R=R, KB=KB, S=S,
                Ls=Ls, offs=offs)
    return in_maps, meta


# ----------------------------------------------------------------------------
# Device program
# ----------------------------------------------------------------------------

def build_program(cfg, meta, sim_mode=False, reps=1, rep_colls=True, phase_reps=None):
    N, IND, HID, HID2, OUT, G, NC = (cfg[k] for k in
                                     ("N", "IND", "HID", "HID2", "OUT",
                                      "G", "NCORES"))
    PB, PCP, TR, R, KB, S = (meta[k] for k in
                             ("PB", "PCP", "TR", "R", "KB", "S"))
    Ls, offs = meta["Ls"], meta["offs"]
    V1 = HID + 2        # phase-A psum width  [h0:126 | ss | h126:128 | sd]
    V2 = HID2 + 2

    ndev = 1 if sim_mode else NC
    nc = bacc.Bacc("TRN2", target_bir_lowering=False, debug=False,
                   num_devices=ndev)

    xT_d = nc.declare_dram_parameter("xT", [128, PB, KB * 128], F8E4, False)
    W1_d = nc.declare_dram_parameter("W1u", [KB, 128, V1], BF16, False)
    W2_d = nc.declare_dram_parameter("W2u", [HID, V2], BF16, False)
    b1v_d = nc.declare_dram_parameter("b1v", [1, HID], F32, False)
    b2v_d = nc.declare_dram_parameter("b2v", [1, HID2], F32, False)
    iotaG_d = nc.declare_dram_parameter("iotaG", [128, G], F32, False)
    Wl_d = nc.declare_dram_parameter("WlBl", [HID2 + 1, OUT], F32, False)
    bgq_d = nc.declare_dram_parameter("bgq", [128, PB], F32, False)
    ivc_d = nc.declare_dram_parameter("ivc", [128, PB], F32, False)
    widx_d = nc.declare_dram_parameter("widx", [16, S * GCHUNK], I16, False)
    sent1_d = nc.declare_dram_parameter("sent1", [1, R], BF16, False)
    sent2_d = nc.declare_dram_parameter("sent2", [1, R], BF16, False)
    out_d = nc.declare_dram_parameter("out", [G, OUT], F32, True)

    shared = dict(addr_space="Shared") if (not sim_mode and NC > 4) else {}
    T1shard = nc.dram_tensor("T1shard", [PCP, R], BF16)
    T1full = nc.dram_tensor("T1full", [TR, R], BF16, **shared)
    T2shard = nc.dram_tensor("T2shard", [PCP, R], BF16)
    T2full = nc.dram_tensor("T2full", [TR, R], BF16, **shared)
    poolin = nc.dram_tensor("poolin", [G, HID2], F32)
    poolout = nc.dram_tensor("poolout", [G, HID2], F32, **shared)

    groups = [list(range(NC))]

    nidx_regs = {}

    def nidx_reg(v):
        if v not in nidx_regs:
            nidx_regs[v] = nc.gpsimd.to_reg(v)
        return nidx_regs[v]

    def gather_block(Gt, Tfull, widx_sb, b, dst0=0):
        L = Ls[b]
        c0 = 0
        while c0 < L:
            CB = min(GCHUNK, L - c0)
            Gvc = Gt[:, (dst0 + c0) * R:(dst0 + c0 + CB) * R].rearrange(
                "p (c e) -> p c e", e=R)
            w0 = (offs[b] + c0) * GCHUNK
            nc.gpsimd.dma_gather(
                out_ap=Gvc, in_ap=Tfull[:],
                idxs_ap=widx_sb[:, w0:w0 + CB * GCHUNK],
                num_idxs=CB * 128, num_idxs_reg=nidx_reg(CB * 128),
                elem_size=R)
            c0 += CB

    with TileContext(nc) as tc:
        with (
            tc.tile_pool(name="const", bufs=1) as cp,
            tc.tile_pool(name="work", bufs=3) as wp,
            tc.tile_pool(name="gath", bufs=2) as gp,
            tc.tile_pool(name="rsp", bufs=1) as rsp,
            tc.tile_pool(name="psA", bufs=2, space="PSUM") as psA,
            tc.tile_pool(name="psB", bufs=2, space="PSUM") as psB,
            tc.tile_pool(name="psP", bufs=1, space="PSUM") as psP,
        ):
            # ---------------- constants to SBUF ----------------
            W1_sb = cp.tile([128, KB * V1], BF16, tag="w1")
            W1v = W1_sb[:].rearrange("p (k h) -> p k h", h=V1)
            nc.sync.dma_start(out=W1v, in_=W1_d[:].rearrange("k p h -> p k h"))
            W2_sb = cp.tile([HID, V2], BF16, tag="w2")
            nc.sync.dma_start(out=W2_sb[:], in_=W2_d[:])
            b1r_sb = cp.tile([128, HID], F32, tag="b1r")
            nc.sync.dma_start(out=b1r_sb[0:1, :], in_=b1v_d[:])
            nc.gpsimd.partition_broadcast(b1r_sb[:], b1r_sb[0:1, :])
            b2r_sb = cp.tile([128, HID2], F32, tag="b2r")
            nc.sync.dma_start(out=b2r_sb[0:1, :], in_=b2v_d[:])
            nc.gpsimd.partition_broadcast(b2r_sb[:], b2r_sb[0:1, :])
            Wl_sb = cp.tile([HID2 + 1, OUT], F32, tag="wl")
            nc.sync.dma_start(out=Wl_sb[:], in_=Wl_d[:])
            # pooling one-hot built on device: P[p, c, g] = (bgq==g) * ivc
            iota_sb = cp.tile([128, G], F32, tag="iog")
            nc.sync.dma_start(out=iota_sb[:], in_=iotaG_d[:])
            bgq_sb = cp.tile([128, PB], F32, tag="bgq")
            nc.sync.dma_start(out=bgq_sb[:], in_=bgq_d[:])
            ivc_sb = cp.tile([128, PB], F32, tag="ivc")
            nc.sync.dma_start(out=ivc_sb[:], in_=ivc_d[:])
            Peq = cp.tile([128, PB * G], BF16, tag="peq")
            nc.vector.tensor_tensor(
                out=Peq[:].rearrange("p (c g) -> p c g", g=G),
                in0=iota_sb[:].rearrange("p (o g) -> p o g", o=1)
                .to_broadcast([128, PB, G]),
                in1=bgq_sb[:, :, None].to_broadcast([128, PB, G]),
                op=ALU.is_equal)
            P_sb = cp.tile([128, PB * G], BF16, tag="pp")
            nc.vector.tensor_tensor(
                out=P_sb[:].rearrange("p (c g) -> p c g", g=G),
                in0=Peq[:].rearrange("p (c g) -> p c g", g=G),
                in1=ivc_sb[:, :, None].to_broadcast([128, PB, G]),
                op=ALU.mult)
            widx_sb = cp.tile([128, S * GCHUNK], I16, tag="widx")
            nc.sync.dma_start(out=widx_sb[0:16, :], in_=widx_d[:])
            nc.sync.dma_start(out=widx_sb[16:32, :], in_=widx_sb[0:16, :])
            nc.sync.dma_start(out=widx_sb[32:64, :], in_=widx_sb[0:32, :])
            nc.sync.dma_start(out=widx_sb[64:128, :], in_=widx_sb[0:64, :])
            xall = cp.tile([128, PB * KB * 128], F8E4, tag="xall")
            nc.sync.dma_start(
                out=xall[:].rearrange("p (c k) -> p c k", k=KB * 128),
                in_=xT_d[:])
            identF = cp.tile([G, G], F32, tag="idf")
            make_identity(nc, identF[:])

            acc1 = cp.tile([128, PB * HID], F32, tag="acc1")
            ob1 = cp.tile([128, PB * HID], F32, tag="ob1")
            r1ball = cp.tile([128, PB * HID], BF16, tag="r1ball")
            dens1 = cp.tile([128, PB], F32, tag="dens1")
            rec1 = cp.tile([128, PB], F32, tag="rec1")
            acc2 = cp.tile([128, PB * HID2], F32, tag="acc2")
            ob2all = cp.tile([128, PB * HID2], F32, tag="ob2all")
            r2ball = cp.tile([128, PB * HID2], BF16, tag="r2ball")
            dens2 = cp.tile([128, PB], F32, tag="dens2")
            rec2 = cp.tile([128, PB], F32, tag="rec2")
            T1sb = cp.tile([128, PB * R], BF16, tag="t1")
            T1sb8 = T1sb[:].bitcast(F8E4)        # [128, PB*256] fp8 view
            T2sb = cp.tile([128, PB * R], BF16, tag="t2")
            nc.vector.memset(T2sb[:], 0.0)
            sd1 = cp.tile([128, PB], F32, tag="sd1")

            pra, prb, prc = phase_reps or (reps, reps, reps)
            for _rep in range(max(pra, prb, prc)):
                # ------- phase A: [h|ss|sd] per node, build T1 rows -------
                for c in range(PB if _rep < pra else 0):
                    ph = psA.tile([128, V1], F32, tag="acc")
                    for kb in range(KB):
                        nc.tensor.matmul(
                            ph[:],
                            lhsT=xall[:, (c * KB + kb) * 128:
                                      (c * KB + kb + 1) * 128],
                            rhs=W1_sb[:, kb * V1:(kb + 1) * V1],
                            start=(kb == 0), stop=(kb == KB - 1),
                        )
                    nc.vector.tensor_copy(
                        T1sb[:, c * R:c * R + 127], ph[:, 0:127])
                    nc.vector.tensor_copy(
                        T1sb8[:, c * 256 + 254:c * 256 + 256],
                        ph[:, 127:129])
                    nc.vector.tensor_copy(sd1[:, c:c + 1], ph[:, 129:130])
                nc.sync.dma_start(
                    out=T1shard[:].rearrange("(c p) w -> p c w", p=128),
                    in_=T1sb[:].rearrange("p (c w) -> p c w", w=R))

                if _rep == 0 or rep_colls:
                    nc.sync.dma_start(out=T1full[TR - 1:TR, :],
                                      in_=sent1_d[:])
                    if sim_mode:
                        nc.sync.dma_start(out=T1full[0:PCP, :],
                                          in_=T1shard[:])
                    else:
                        nc.gpsimd.collective_compute(
                            "AllGather", ALU.bypass, replica_groups=groups,
                            ins=[T1shard[:]], outs=[T1full[0:TR - 1, :]])

                # ------------- phase B: GAT layer 1 + T2 build -------------
                nb = PB if _rep < prb else 0
                for b in range(0, nb, 2):
                    L = Ls[b]
                    Gt = gp.tile([128, 2 * L * R], BF16, tag="g1")
                    Gv = Gt[:].rearrange("p (l w) -> p l w", w=R)
                    Gt8 = Gt[:].bitcast(F8E4).rearrange(
                        "p (l w) -> p l w", w=256)
                    gather_block(Gt, T1full, widx_sb, b)
                    gather_block(Gt, T1full, widx_sb, b + 1, dst0=L)
                    # e = lrelu(ss_src + sd_dst); w = exp(e); den = sum_j w
                    t_t = wp.tile([128, 2 * L], F32, tag="tpre")
                    nc.vector.tensor_tensor(
                        out=t_t[:].rearrange("p (b l) -> p b l", l=L),
                        in0=Gv[:, :, 126].rearrange("p (b l) -> p b l", l=L),
                        in1=sd1[:, b:b + 2, None].to_broadcast([128, 2, L]),
                        op=ALU.add)
                    wl_t = wp.tile([128, 2 * L], F32, tag="wl1")
                    nc.vector.scalar_tensor_tensor(
                        out=wl_t[:], in0=t_t[:], scalar=NEG_SLOPE,
                        in1=t_t[:], op0=ALU.mult, op1=ALU.max)
                    wex = wp.tile([128, 2 * L], F32, tag="we1")
                    nc.scalar.activation(wex[:], wl_t[:], AF.Exp)
                    nc.vector.tensor_reduce(
                        out=dens1[:, b:b + 2],
                        in_=wex[:].rearrange("p (b l) -> p b l", l=L),
                        axis=mybir.AxisListType.X, op=ALU.add)
                    # weighted rows + reduction over slots
                    rs = rsp.tile([128, 2 * L * R], BF16, tag="rs")
                    rs3 = rs[:].rearrange("p (l w) -> p l w", w=R)
                    nc.vector.tensor_tensor(
                        out=rs3[:, :, 0:126], in0=Gv[:, :, 0:126],
                        in1=wex[:, :, None].to_broadcast([128, 2 * L, 126]),
                        op=ALU.mult)
                    nc.vector.tensor_tensor(
                        out=rs3[:, :, 126:128], in0=Gt8[:, :, 254:256],
                        in1=wex[:, :, None].to_broadcast([128, 2 * L, 2]),
                        op=ALU.mult)
                    nc.vector.tensor_reduce(
                        out=acc1[:, b * HID:(b + 2) * HID]
                        .rearrange("p (b f) -> p b f", f=HID),
                        in_=rs[:].rearrange("p (b l f) -> p b f l", f=R, l=L),
                        axis=mybir.AxisListType.X, op=ALU.add)
                if nb:
                    nc.vector.reciprocal(rec1[:], dens1[:])
                    for b in range(nb):
                        nc.vector.scalar_tensor_tensor(
                            out=ob1[:, b * HID:(b + 1) * HID],
                            in0=acc1[:, b * HID:(b + 1) * HID],
                            scalar=rec1[:, b:b + 1],
                            in1=b1r_sb[:], op0=ALU.mult, op1=ALU.add)
                    nc.vector.tensor_scalar(
                        out=r1ball[:], in0=ob1[:], scalar1=0.0, scalar2=None,
                        op0=ALU.max)
                    for b in range(nb):
                        r1T = wp.tile([128, HID], BF16, tag="r1T")
                        nc.sync.dma_start_transpose(
                            r1T[:], r1ball[:, b * HID:(b + 1) * HID])
                        ph2 = psB.tile([128, V2], F32, tag="tr")
                        nc.tensor.matmul(ph2[:], lhsT=r1T[:], rhs=W2_sb[:],
                                         start=True, stop=True)
                        nc.vector.tensor_copy(
                            T2sb[:, b * R:b * R + V2], ph2[:])
                nc.sync.dma_start(
                    out=T2shard[:].rearrange("(c p) w -> p c w", p=128),
                    in_=T2sb[:].rearrange("p (c w) -> p c w", w=R))

                if _rep == 0 or rep_colls:
                    nc.sync.dma_start(out=T2full[TR - 1:TR, :],
                                      in_=sent2_d[:])
                    if sim_mode:
                        nc.sync.dma_start(out=T2full[0:PCP, :],
                                          in_=T2shard[:])
                    else:
                        nc.gpsimd.collective_compute(
                            "AllGather", ALU.bypass, replica_groups=groups,
                            ins=[T2shard[:]], outs=[T2full[0:TR - 1, :]])

                # ------------- phase C: GAT layer 2 + pooling -------------
                if _rep == 0:
                    pool_ps = psP.tile([G, HID2], F32, tag="pool")
                ncb = PB if _rep < prc else 0
                for b in range(0, ncb, 2):
                    L = Ls[b]
                    Gt = gp.tile([128, 2 * L * R], BF16, tag="g1")
                    Gv = Gt[:].rearrange("p (l w) -> p l w", w=R)
                    gather_block(Gt, T2full, widx_sb, b)
                    gather_block(Gt, T2full, widx_sb, b + 1, dst0=L)
                    t_t = wp.tile([128, 2 * L], F32, tag="tpre")
                    nc.vector.tensor_tensor(
                        out=t_t[:].rearrange("p (b l) -> p b l", l=L),
                        in0=Gv[:, :, HID2].rearrange("p (b l) -> p b l", l=L),
                        in1=T2sb[:].rearrange("p (c w) -> p c w", w=R)
                        [:, b:b + 2, HID2 + 1:HID2 + 2]
                        .to_broadcast([128, 2, L]),
                        op=ALU.add)
                    wl_t = wp.tile([128, 2 * L], F32, tag="wl1")
                    nc.vector.scalar_tensor_tensor(
                        out=wl_t[:], in0=t_t[:], scalar=NEG_SLOPE,
                        in1=t_t[:], op0=ALU.mult, op1=ALU.max)
                    wex = wp.tile([128, 2 * L], F32, tag="we1")
                    nc.scalar.activation(wex[:], wl_t[:], AF.Exp)
                    nc.vector.tensor_reduce(
                        out=dens2[:, b:b + 2],
                        in_=wex[:].rearrange("p (b l) -> p b l", l=L),
                        axis=mybir.AxisListType.X, op=ALU.add)
                    rs = rsp.tile([128, 2 * L * HID2], BF16, tag="rs")
                    rs3 = rs[:].rearrange("p (l w) -> p l w", w=HID2)
                    nc.vector.tensor_tensor(
                        out=rs3[:], in0=Gv[:, :, 0:HID2],
                        in1=wex[:, :, None].to_broadcast([128, 2 * L, HID2]),
                        op=ALU.mult)
                    nc.vector.tensor_reduce(
                        out=acc2[:, b * HID2:(b + 2) * HID2]
                        .rearrange("p (b f) -> p b f", f=HID2),
                        in_=rs[:].rearrange("p (b l f) -> p b f l",
                                            f=HID2, l=L),
                        axis=mybir.AxisListType.X, op=ALU.add)
                if ncb:
                    nc.vector.reciprocal(rec2[:], dens2[:])
                    for b in range(ncb):
                        nc.vector.scalar_tensor_tensor(
                            out=ob2all[:, b * HID2:(b + 1) * HID2],
                            in0=acc2[:, b * HID2:(b + 1) * HID2],
                            scalar=rec2[:, b:b + 1],
                            in1=b2r_sb[:], op0=ALU.mult, op1=ALU.add)
                    nc.vector.tensor_scalar(
                        out=r2ball[:], in0=ob2all[:], scalar1=0.0,
                        scalar2=None, op0=ALU.max)
                    for b in range(ncb):
                        nc.tensor.matmul(
                            pool_ps[:], lhsT=P_sb[:, b * G:(b + 1) * G],
                            rhs=r2ball[:, b * HID2:(b + 1) * HID2],
                            start=(b == 0), stop=(b == PB - 1))

            pooled = wp.tile([G, HID2], F32, tag="pool")
            nc.vector.tensor_copy(pooled[:], pool_ps[:])
            nc.sync.dma_start(out=poolin[:], in_=pooled[:])
            if sim_mode:
                nc.sync.dma_start(out=poolout[:], in_=poolin[:])
            else:
                nc.gpsimd.collective_compute(
                    "AllReduce", ALU.add, replica_groups=groups,
                    ins=[poolin[:]], outs=[poolout[:]])
            pooled_r = wp.tile([G, HID2], F32, tag="poolr")
            nc.sync.dma_start(out=pooled_r[:], in_=poolout[:])
            pTf = psB.tile([HID2, G], F32, tag="tr")
            nc.tensor.transpose(pTf[:], pooled_r[:], identity=identF[:])
            fin = wp.tile([HID2 + 1, G], F32, tag="fin")
            nc.vector.tensor_copy(fin[:HID2, :], pTf[:])
            nc.vector.memset(fin[HID2:HID2 + 1, :], 1.0)
            out_ps = psB.tile([G, OUT], F32, tag="tr")
            nc.tensor.matmul(out_ps[:], lhsT=fin[:], rhs=Wl_sb[:],
                             start=True, stop=True)
            out_sb = wp.tile([G, OUT], F32, tag="outsb")
            nc.vector.tensor_copy(out_sb[:], out_ps[:])
            nc.sync.dma_start(out=out_d[:], in_=out_sb[:])

    nc.compile()
    return nc


# ----------------------------------------------------------------------------
# Entry point
# ----------------------------------------------------------------------------

LAST_RESULTS = None


def kernel(**inputs):
    global LAST_RESULTS
    cfg = full_cfg()
    in_maps, meta = preprocess(cfg=cfg, **inputs)
    nc = build_program(cfg, meta)
    res = run_bass_kernel_spmd(nc, in_maps, core_ids=list(range(cfg["NCORES"])))
    LAST_RESULTS = res
    return np.asarray(res.results[0]["out"], np.float32)
